# revision 1
# baseline (speedup 1.0000x reference)
"""GATv2WithGlobal Trainium2 kernel — 8-core SPMD bass implementation.

Strategy (dst-sharded message passing):
- Nodes padded 30000->30720, sharded as 8 cores x 30 blocks x 128 dst nodes.
- Edges (+self loops) sorted by dst, grouped per dst-block, padded to a uniform
  t_max tiles of 128 edges per block (SPMD needs one program for all cores).
- Per edge tile: indirect-DMA gather of source features, one-hot matmuls for
  target-feature broadcast and scatter-aggregation, PSUM accumulation,
  LeakyReLU on ScalarE, attention scores via broadcast-mul + strided reduce on
  VectorE, exp on ScalarE. Segment softmax without max-subtraction (scores are
  O(few), fp32-safe); denominator aggregated as a 4-column matmul.
- BatchNorm via E[x^2]-mu^2 with the layer bias folded into the stats
  (variance is bias-invariant); BN is applied with broadcast scale/shift.
- Cross-core exchanges: AllReduce for BN stats & pooled features, AllGather for
  the layer-2 source transform table.
"""

import numpy as np

import concourse.bass as bass
import concourse.mybir as mybir
import concourse.tile as tile
from concourse import bacc
from concourse.bass_utils import run_bass_kernel_spmd

# problem dims (hardcoded per contract)
N = 30000
N_PAD = 30720
P = 128
N_CORES = 8
NBLK = N_PAD // P            # 240
NBLK_CORE = NBLK // N_CORES  # 30
NSHARD = NBLK_CORE * P       # 3840
H, C, HC = 4, 128, 512
F_IN, G_DIM, B = 9, 50, 64
SLOPE = 0.2
EPS_BN = 1e-5

F32 = mybir.dt.float32
BF16 = mybir.dt.bfloat16
I32 = mybir.dt.int32
AF = mybir.ActivationFunctionType
OP = mybir.AluOpType

_PROGRAM_CACHE: dict = {}


def _edge_layer(nc, tc, ctx_pools, t_max, am_dram, xr_dram, att_sb, raw_dram,
                src_idx_d, oh_d, oht_d, consts, psBN_sum, psBN_sq, abl=()):
    """One GATv2 message-passing layer over this core's 30 dst blocks (bf16).
    Writes aggregated (pre-BN, bias-free) features to raw_dram (f32) and
    accumulates BN sum/sumsq into the two persistent PSUM tiles."""
    identb, ones_row, ones_col = consts
    sb, psA, psC, psD = ctx_pools

    for b in range(NBLK_CORE):
        idx_sb = sb.tile([P, t_max], I32, tag="idx", bufs=2)
        nc.sync.dma_start(idx_sb[:], src_idx_d[b])
        xr_blk = sb.tile([P, HC], BF16, tag="xrb", bufs=2)
        nc.sync.dma_start(xr_blk[:], xr_dram[b * P:(b + 1) * P, :])

        psum_C = psC.tile([P, HC], F32, space="PSUM", tag="C")
        psum_D = psD.tile([P, H], F32, space="PSUM", tag="D")

        for t in range(t_max):
            XL = sb.tile([P, HC], BF16, tag="XL")
            if "nogather" in abl:
                nc.sync.dma_start(XL[:], am_dram[t * P:(t + 1) * P, :])
            else:
                nc.gpsimd.indirect_dma_start(
                    out=XL[:], out_offset=None, in_=am_dram[:],
                    in_offset=bass.IndirectOffsetOnAxis(ap=idx_sb[:, t:t + 1],
                                                        axis=0),
                )
            OHT_t = sb.tile([P, P], BF16, tag="OHT")
            nc.sync.dma_start(OHT_t[:], oht_d[b, t])
            OH_t = sb.tile([P, P], BF16, tag="OH")
            nc.sync.dma_start(OH_t[:], oh_d[b, t])

            psum_m = psA.tile([P, HC], F32, space="PSUM", tag="m")
            nc.tensor.matmul(psum_m[:], lhsT=identb[:], rhs=XL[:], start=True,
                             stop=False)
            nc.tensor.matmul(psum_m[:], lhsT=OHT_t[:], rhs=xr_blk[:],
                             start=False, stop=True)
            LR = sb.tile([P, HC], BF16, tag="LR")
            nc.scalar.activation(LR[:], psum_m[:], AF.Prelu, alpha=SLOPE)
            S = sb.tile([P, H], F32, tag="S")
            if "noscore" in abl:
                nc.vector.memset(S[:], 0.5)
            else:
                TM = sb.tile([P, HC], BF16, tag="TM")
                nc.vector.tensor_mul(TM[:], LR[:], att_sb[:])
                nc.vector.tensor_reduce(
                    out=S[:], in_=TM[:].rearrange("p (h c) -> p h c", h=H),
                    axis=mybir.AxisListType.X, op=OP.add)
            P4f = sb.tile([P, H], F32, tag="P4f")
            nc.scalar.activation(P4f[:], S[:], AF.Exp)
            P4b = sb.tile([P, H], BF16, tag="P4b")
            nc.vector.tensor_copy(P4b[:], P4f[:])
            XLP = sb.tile([P, HC], BF16, tag="XLP")
            for h in range(H):
                nc.vector.tensor_scalar_mul(XLP[:, h * C:(h + 1) * C],
                                            XL[:, h * C:(h + 1) * C],
                                            P4f[:, h:h + 1])
            nc.tensor.matmul(psum_C[:], lhsT=OH_t[:], rhs=XLP[:], start=(t == 0),
                             stop=(t == t_max - 1))
            nc.tensor.matmul(psum_D[:], lhsT=OH_t[:], rhs=P4b[:], start=(t == 0),
                             stop=(t == t_max - 1))

        # block flush: OUT = C / (D + eps); BN moment accumulation
        Deps = sb.tile([P, H], F32, tag="Deps")
        nc.vector.tensor_scalar_add(Deps[:], psum_D[:], 1e-16)
        rec = sb.tile([P, H], F32, tag="rec")
        nc.vector.reciprocal(rec[:], Deps[:])
        OUT = sb.tile([P, HC], F32, tag="OUT")
        for h in range(H):
            nc.vector.tensor_scalar_mul(OUT[:, h * C:(h + 1) * C],
                                        psum_C[:, h * C:(h + 1) * C],
                                        rec[:, h:h + 1])
        nc.sync.dma_start(raw_dram[b * P:(b + 1) * P, :], OUT[:])
        SQ = sb.tile([P, HC], F32, tag="SQ")
        nc.scalar.activation(SQ[:], OUT[:], AF.Square)
        nc.tensor.matmul(psBN_sum[:], lhsT=ones_col[:], rhs=OUT[:],
                         start=(b == 0), stop=(b == NBLK_CORE - 1))
        nc.tensor.matmul(psBN_sq[:], lhsT=ones_col[:], rhs=SQ[:],
                         start=(b == 0), stop=(b == NBLK_CORE - 1))


def _bn_scale_shift(nc, hold, sb, psum_pool, stats_in_d, stats_out_d, psBN_sum,
                    psBN_sq, bng_row_d, bnb_row_d, bias_row_d, consts, tag,
                    collective_fn=None):
    """AllReduce BN moments across cores, compute broadcast scale/shift tiles.
    Small temps go in `sb` (transient pool); the returned broadcast tiles
    (scale_bc, shift_bc) [128, 512] live in `hold`."""
    ident, ones_row, ones_col = consts
    stats = sb.tile([1, 2 * HC], F32, tag=f"st{tag}", bufs=1)
    nc.scalar.copy(stats[:, :HC], psBN_sum[:])
    nc.scalar.copy(stats[:, HC:], psBN_sq[:])
    nc.sync.dma_start(stats_in_d[:], stats[:])
    collective_fn("AllReduce", OP.add, [list(range(N_CORES))],
                  [stats_in_d[:]], [stats_out_d[:]])
    st = sb.tile([1, 2 * HC], F32, tag=f"str{tag}", bufs=1)
    nc.sync.dma_start(st[:], stats_out_d[:])

    bng = sb.tile([1, HC], F32, tag=f"bng{tag}", bufs=1)
    nc.sync.dma_start(bng[:], bng_row_d[:])
    bnb = sb.tile([1, HC], F32, tag=f"bnb{tag}", bufs=1)
    nc.sync.dma_start(bnb[:], bnb_row_d[:])
    bias = sb.tile([1, HC], F32, tag=f"bias{tag}", bufs=1)
    nc.sync.dma_start(bias[:], bias_row_d[:])

    inv_n = 1.0 / N
    mu0 = sb.tile([1, HC], F32, tag=f"mu0{tag}", bufs=1)
    nc.vector.tensor_scalar_mul(mu0[:], st[:, :HC], inv_n)
    ex2 = sb.tile([1, HC], F32, tag=f"ex2{tag}", bufs=1)
    nc.vector.tensor_scalar_mul(ex2[:], st[:, HC:], inv_n)
    mu0sq = sb.tile([1, HC], F32, tag=f"mu0sq{tag}", bufs=1)
    nc.vector.tensor_mul(mu0sq[:], mu0[:], mu0[:])
    var = sb.tile([1, HC], F32, tag=f"var{tag}", bufs=1)
    nc.vector.tensor_sub(var[:], ex2[:], mu0sq[:])
    vareps = sb.tile([1, HC], F32, tag=f"vareps{tag}", bufs=1)
    nc.vector.tensor_scalar_add(vareps[:], var[:], EPS_BN)
    sd = sb.tile([1, HC], F32, tag=f"sd{tag}", bufs=1)
    nc.scalar.activation(sd[:], vareps[:], AF.Sqrt)
    rsd = sb.tile([1, HC], F32, tag=f"rsd{tag}", bufs=1)
    nc.vector.reciprocal(rsd[:], sd[:])
    scale = sb.tile([1, HC], F32, tag=f"scale{tag}", bufs=1)
    nc.vector.tensor_mul(scale[:], bng[:], rsd[:])
    mup = sb.tile([1, HC], F32, tag=f"mup{tag}", bufs=1)
    nc.vector.tensor_add(mup[:], mu0[:], bias[:])
    t1 = sb.tile([1, HC], F32, tag=f"t1{tag}", bufs=1)
    nc.vector.tensor_mul(t1[:], mup[:], scale[:])
    shift = sb.tile([1, HC], F32, tag=f"shift{tag}", bufs=1)
    nc.vector.tensor_sub(shift[:], bnb[:], t1[:])

    ps_s = psum_pool.tile([P, HC], F32, space="PSUM", tag="m")
    nc.tensor.matmul(ps_s[:], lhsT=ones_row[:], rhs=scale[:], start=True,
                     stop=True)
    scale_bc = hold.tile([P, HC], F32, tag=f"scbc{tag}")
    nc.scalar.copy(scale_bc[:], ps_s[:])
    ps_h = psum_pool.tile([P, HC], F32, space="PSUM", tag="m")
    nc.tensor.matmul(ps_h[:], lhsT=ones_row[:], rhs=shift[:], start=True,
                     stop=True)
    shift_bc = hold.tile([P, HC], F32, tag=f"shbc{tag}")
    nc.scalar.copy(shift_bc[:], ps_h[:])
    return scale_bc, shift_bc


def _build_program(t_max, sim_mode=False, abl=(), ebufs=6, mbufs=4, dbbufs=0):
    nc = bacc.Bacc("TRN2", target_bir_lowering=False, debug=False,
                   num_devices=1 if sim_mode else N_CORES)

    def _collective(kind, op, groups_, ins, outs):
        if sim_mode:
            if kind == "AllGather":
                o = outs[0]
                n_in = ins[0].shape[0]
                nc.sync.dma_start(o.tensor[0:n_in, :], ins[0])
            else:
                nc.sync.dma_start(outs[0], ins[0])
        else:
            nc.gpsimd.collective_compute(kind, op, replica_groups=groups_,
                                         ins=ins, outs=outs)

    # ---- I/O declarations -------------------------------------------------
    xT_aug_d = nc.dram_tensor("xT_aug", [F_IN + 1, N_PAD], F32, kind="ExternalInput")
    xTq_aug_d = nc.dram_tensor("xTq_aug", [F_IN + 1, NSHARD], F32, kind="ExternalInput")
    W1l_d = nc.dram_tensor("W1l_aug", [F_IN + 1, HC], F32, kind="ExternalInput")
    W1r_d = nc.dram_tensor("W1r_aug", [F_IN + 1, HC], F32, kind="ExternalInput")
    W2l_d = nc.dram_tensor("W2l", [HC, HC], F32, kind="ExternalInput")
    W2r_d = nc.dram_tensor("W2r", [HC, HC], F32, kind="ExternalInput")
    b2l_d = nc.dram_tensor("b2l_row", [1, HC], F32, kind="ExternalInput")
    b2r_d = nc.dram_tensor("b2r_row", [1, HC], F32, kind="ExternalInput")
    att1_d = nc.dram_tensor("att1_bc", [P, HC], BF16, kind="ExternalInput")
    att2_d = nc.dram_tensor("att2_bc", [P, HC], BF16, kind="ExternalInput")
    bn1g_d = nc.dram_tensor("bn1_g_row", [1, HC], F32, kind="ExternalInput")
    bn1b_d = nc.dram_tensor("bn1_b_row", [1, HC], F32, kind="ExternalInput")
    bias1_d = nc.dram_tensor("bias1_row", [1, HC], F32, kind="ExternalInput")
    bn2g_d = nc.dram_tensor("bn2_g_row", [1, HC], F32, kind="ExternalInput")
    bn2b_d = nc.dram_tensor("bn2_b_row", [1, HC], F32, kind="ExternalInput")
    bias2_d = nc.dram_tensor("bias2_row", [1, HC], F32, kind="ExternalInput")
    fc1_d = nc.dram_tensor("fc1_aug", [HC + G_DIM + 1, C], F32, kind="ExternalInput")
    fc2_d = nc.dram_tensor("fc2_w", [C, 1], F32, kind="ExternalInput")
    fc2b_d = nc.dram_tensor("fc2_b_col", [B, 1], F32, kind="ExternalInput")
    gfT_d = nc.dram_tensor("gfT", [G_DIM, B], F32, kind="ExternalInput")
    cntinv_d = nc.dram_tensor("cntinv_row", [1, B], F32, kind="ExternalInput")
    ident_d = nc.dram_tensor("ident", [P, P], F32, kind="ExternalInput")
    ones_row_d = nc.dram_tensor("ones_row", [1, P], F32, kind="ExternalInput")
    ones_col_d = nc.dram_tensor("ones_col", [P, 1], F32, kind="ExternalInput")
    src_idx_d = nc.dram_tensor("src_idx", [NBLK_CORE, P, t_max], I32, kind="ExternalInput")
    oh_d = nc.dram_tensor("oh", [NBLK_CORE, t_max, P, P], BF16, kind="ExternalInput")
    oht_d = nc.dram_tensor("oht", [NBLK_CORE, t_max, P, P], BF16, kind="ExternalInput")
    identb_d = nc.dram_tensor("identb", [P, P], BF16, kind="ExternalInput")
    ohb_d = nc.dram_tensor("ohb", [NBLK_CORE, P, B], F32, kind="ExternalInput")

    out_d = nc.dram_tensor("out_final", [B, 1], F32, kind="ExternalOutput")

    # internal DRAM
    am1_d = nc.dram_tensor("am1", [N_PAD, HC], BF16)
    xr1_d = nc.dram_tensor("xr1", [NSHARD, HC], BF16)
    xr2_d = nc.dram_tensor("xr2", [NSHARD, HC], BF16)
    hT_d = nc.dram_tensor("hT", [HC, NSHARD], F32)
    h1raw_d = nc.dram_tensor("h1raw", [NSHARD, HC], F32)
    am2s_d = nc.dram_tensor("am2s", [NSHARD, HC], BF16)
    am2_d = nc.dram_tensor("am2", [N_PAD, HC], BF16, addr_space="Shared")
    h2raw_d = nc.dram_tensor("h2raw", [NSHARD, HC], F32)
    bn1in_d = nc.dram_tensor("bn1in", [1, 2 * HC], F32)
    bn1out_d = nc.dram_tensor("bn1out", [1, 2 * HC], F32, addr_space="Shared")
    bn2in_d = nc.dram_tensor("bn2in", [1, 2 * HC], F32)
    bn2out_d = nc.dram_tensor("bn2out", [1, 2 * HC], F32, addr_space="Shared")
    poolin_d = nc.dram_tensor("poolin", [H, P, B], F32)
    poolout_d = nc.dram_tensor("poolout", [H, P, B], F32, addr_space="Shared")

    groups = [list(range(N_CORES))]

    with tile.TileContext(nc) as tc:
        with (
            tc.tile_pool(name="const", bufs=1) as cpool,
            tc.tile_pool(name="hold", bufs=1) as hold,
        ):
            # constants
            ident = cpool.tile([P, P], F32)
            nc.sync.dma_start(ident[:], ident_d[:])
            identb = cpool.tile([P, P], BF16)
            nc.sync.dma_start(identb[:], identb_d[:])
            ones_row = cpool.tile([1, P], F32)
            nc.sync.dma_start(ones_row[:], ones_row_d[:])
            ones_col = cpool.tile([P, 1], F32)
            nc.sync.dma_start(ones_col[:], ones_col_d[:])
            att1 = cpool.tile([P, HC], BF16)
            nc.sync.dma_start(att1[:], att1_d[:])
            att2 = cpool.tile([P, HC], BF16)
            nc.sync.dma_start(att2[:], att2_d[:])
            consts = (identb, ones_row, ones_col)
            bn_consts = (ident, ones_row, ones_col)


            # ---- P1: layer-1 node transforms --------------------------------
            with (
                tc.tile_pool(name="p1sb", bufs=3) as p1sb,
                tc.tile_pool(name="p1ps", bufs=4, space="PSUM") as p1ps,
            ):
                W1l = p1sb.tile([F_IN + 1, HC], F32, bufs=1)
                nc.sync.dma_start(W1l[:], W1l_d[:])
                W1r = p1sb.tile([F_IN + 1, HC], F32, bufs=1)
                nc.sync.dma_start(W1r[:], W1r_d[:])
                # am1 for ALL nodes (replicated on every core)
                for mt in range(NBLK):
                    xt = p1sb.tile([F_IN + 1, P], F32, tag="xt")
                    nc.sync.dma_start(xt[:], xT_aug_d[:, mt * P:(mt + 1) * P])
                    ps = p1ps.tile([P, HC], F32, space="PSUM", tag="p1")
                    nc.tensor.matmul(ps[:], lhsT=xt[:], rhs=W1l[:], start=True,
                                     stop=True)
                    ev = p1sb.tile([P, HC], BF16, tag="ev")
                    nc.scalar.copy(ev[:], ps[:])
                    nc.sync.dma_start(am1_d[mt * P:(mt + 1) * P, :], ev[:])
                # xr1 for own shard only -> SBUF resident
                for j in range(NBLK_CORE):
                    xtq = p1sb.tile([F_IN + 1, P], F32, tag="xtq")
                    nc.sync.dma_start(xtq[:], xTq_aug_d[:, j * P:(j + 1) * P])
                    ps = p1ps.tile([P, HC], F32, space="PSUM", tag="p1")
                    nc.tensor.matmul(ps[:], lhsT=xtq[:], rhs=W1r[:], start=True,
                                     stop=True)
                    ev2 = p1sb.tile([P, HC], BF16, tag="ev2")
                    nc.scalar.copy(ev2[:], ps[:])
                    nc.sync.dma_start(xr1_d[j * P:(j + 1) * P, :], ev2[:])

            # ---- P2: layer-1 edge aggregation -------------------------------
            with (
                tc.tile_pool(name="e1sb", bufs=ebufs) as esb,
                tc.tile_pool(name="e1psA", bufs=mbufs, space="PSUM") as psA,
                tc.tile_pool(name="e1psC", bufs=1, space="PSUM") as psC,
                tc.tile_pool(name="e1psD", bufs=1, space="PSUM") as psD,
                tc.tile_pool(name="e1psBN", bufs=1, space="PSUM") as psBN,
            ):
                psBN_sum = psBN.tile([1, HC], F32, space="PSUM")
                psBN_sq = psBN.tile([1, HC], F32, space="PSUM")
                _edge_layer(nc, tc, (esb, psA, psC, psD), t_max, am1_d,
                            xr1_d, att1, h1raw_d, src_idx_d, oh_d, oht_d,
                            consts, psBN_sum, psBN_sq, abl)

                # ---- P3: BN1 stats + scale/shift ----------------------------
                scale1_bc, shift1_bc = _bn_scale_shift(
                    nc, hold, esb, psA, bn1in_d, bn1out_d, psBN_sum, psBN_sq,
                    bn1g_d, bn1b_d, bias1_d, bn_consts, "b1", _collective)

            # ---- P4: BN1 apply + relu + build hT ----------------------------
            with (
                tc.tile_pool(name="p4sb", bufs=3) as p4sb,
                tc.tile_pool(name="p4ps", bufs=2, space="PSUM") as p4ps,
            ):
                for j in range(NBLK_CORE):
                    raw = p4sb.tile([P, HC], F32, tag="raw")
                    nc.sync.dma_start(raw[:], h1raw_d[j * P:(j + 1) * P, :])
                    t1 = p4sb.tile([P, HC], F32, tag="t1")
                    nc.vector.tensor_mul(t1[:], raw[:], scale1_bc[:])
                    t2 = p4sb.tile([P, HC], F32, tag="t2")
                    nc.vector.tensor_add(t2[:], t1[:], shift1_bc[:])
                    hsb = p4sb.tile([P, HC], F32, tag="h")
                    nc.vector.tensor_scalar_max(hsb[:], t2[:], 0.0)
                    pst = p4ps.tile([P, HC], F32, space="PSUM", tag="tr")
                    for ch in range(4):
                        nc.tensor.transpose(pst[:, ch * P:(ch + 1) * P],
                                            hsb[:, ch * P:(ch + 1) * P],
                                            ident[:])
                    ev4 = p4sb.tile([P, HC], F32, tag="ev4")
                    nc.scalar.copy(ev4[:], pst[:])
                    for ch in range(4):
                        nc.sync.dma_start(
                            hT_d[ch * P:(ch + 1) * P, j * P:(j + 1) * P],
                            ev4[:, ch * P:(ch + 1) * P])

            # ---- P5: layer-2 node transforms --------------------------------
            with (
                tc.tile_pool(name="p5sb", bufs=3) as p5sb,
                tc.tile_pool(name="p5w", bufs=1) as p5w,
                tc.tile_pool(name="p5ps", bufs=4, space="PSUM") as p5ps,
            ):
                W2l_sb = [p5w.tile([P, HC], F32, name=f"W2l{k}", tag=f"W2l{k}") for k in range(4)]
                W2r_sb = [p5w.tile([P, HC], F32, name=f"W2r{k}", tag=f"W2r{k}") for k in range(4)]
                for k in range(4):
                    nc.sync.dma_start(W2l_sb[k][:], W2l_d[k * P:(k + 1) * P, :])
                    nc.sync.dma_start(W2r_sb[k][:], W2r_d[k * P:(k + 1) * P, :])
                b2l = p5w.tile([1, HC], F32)
                nc.sync.dma_start(b2l[:], b2l_d[:])
                b2r = p5w.tile([1, HC], F32)
                nc.sync.dma_start(b2r[:], b2r_d[:])
                for j in range(NBLK_CORE):
                    hTj = []
                    for k in range(4):
                        hx = p5sb.tile([P, P], F32, tag=f"hTj{k}",
                                       name=f"hTj{k}")
                        nc.sync.dma_start(
                            hx[:], hT_d[k * P:(k + 1) * P, j * P:(j + 1) * P])
                        hTj.append(hx)
                    psl = p5ps.tile([P, HC], F32, space="PSUM", tag="l")
                    for k in range(4):
                        nc.tensor.matmul(psl[:], lhsT=hTj[k][:],
                                         rhs=W2l_sb[k][:], start=(k == 0),
                                         stop=False)
                    nc.tensor.matmul(psl[:], lhsT=ones_row[:], rhs=b2l[:],
                                     start=False, stop=True)
                    ev = p5sb.tile([P, HC], BF16, tag="ev")
                    nc.scalar.copy(ev[:], psl[:])
                    nc.sync.dma_start(am2s_d[j * P:(j + 1) * P, :], ev[:])
                    psr = p5ps.tile([P, HC], F32, space="PSUM", tag="r")
                    for k in range(4):
                        nc.tensor.matmul(psr[:], lhsT=hTj[k][:],
                                         rhs=W2r_sb[k][:], start=(k == 0),
                                         stop=False)
                    nc.tensor.matmul(psr[:], lhsT=ones_row[:], rhs=b2r[:],
                                     start=False, stop=True)
                    ev5 = p5sb.tile([P, HC], BF16, tag="ev5")
                    nc.scalar.copy(ev5[:], psr[:])
                    nc.sync.dma_start(xr2_d[j * P:(j + 1) * P, :], ev5[:])

            # ---- P6: AllGather layer-2 source transforms --------------------
            _collective("AllGather", OP.bypass, groups,
                        [am2s_d[:]], [am2_d[:]])

            # ---- P7: layer-2 edge aggregation -------------------------------
            with (
                tc.tile_pool(name="e2sb", bufs=ebufs) as esb,
                tc.tile_pool(name="e2psA", bufs=mbufs, space="PSUM") as psA,
                tc.tile_pool(name="e2psC", bufs=1, space="PSUM") as psC,
                tc.tile_pool(name="e2psD", bufs=1, space="PSUM") as psD,
                tc.tile_pool(name="e2psBN", bufs=1, space="PSUM") as psBN,
            ):
                psBN_sum = psBN.tile([1, HC], F32, space="PSUM")
                psBN_sq = psBN.tile([1, HC], F32, space="PSUM")
                _edge_layer(nc, tc, (esb, psA, psC, psD), t_max, am2_d,
                            xr2_d, att2, h2raw_d, src_idx_d, oh_d, oht_d,
                            consts, psBN_sum, psBN_sq, abl)
                scale2_bc, shift2_bc = _bn_scale_shift(
                    nc, hold, esb, psA, bn2in_d, bn2out_d, psBN_sum, psBN_sq,
                    bn2g_d, bn2b_d, bias2_d, bn_consts, "b2", _collective)

            # ---- P8: BN2 apply + relu + pooling -----------------------------
            with (
                tc.tile_pool(name="p8sb", bufs=3) as p8sb,
                tc.tile_pool(name="p8ps", bufs=1, space="PSUM") as p8ps,
            ):
                pool_ps = [p8ps.tile([P, B], F32, space="PSUM", name=f"pool{k}", tag=f"pool{k}") for k in range(4)]
                for j in range(NBLK_CORE):
                    raw = p8sb.tile([P, HC], F32, tag="raw")
                    nc.sync.dma_start(raw[:], h2raw_d[j * P:(j + 1) * P, :])
                    t1 = p8sb.tile([P, HC], F32, tag="t1")
                    nc.vector.tensor_mul(t1[:], raw[:], scale2_bc[:])
                    t2 = p8sb.tile([P, HC], F32, tag="t2")
                    nc.vector.tensor_add(t2[:], t1[:], shift2_bc[:])
                    hsb = p8sb.tile([P, HC], F32, tag="h")
                    nc.vector.tensor_scalar_max(hsb[:], t2[:], 0.0)
                    ohb = p8sb.tile([P, B], F32, tag="ohb")
                    nc.sync.dma_start(ohb[:], ohb_d[j])
                    for ch in range(4):
                        nc.tensor.matmul(pool_ps[ch][:],
                                         lhsT=hsb[:, ch * P:(ch + 1) * P],
                                         rhs=ohb[:], start=(j == 0),
                                         stop=(j == NBLK_CORE - 1))
                poolsb = p8sb.tile([P, 4 * B], F32)
                for ch in range(4):
                    nc.scalar.copy(poolsb[:, ch * B:(ch + 1) * B], pool_ps[ch][:])
                for ch in range(4):
                    nc.sync.dma_start(poolin_d[ch], poolsb[:, ch * B:(ch + 1) * B])
                _collective("AllReduce", OP.add, groups,
                            [poolin_d[:]], [poolout_d[:]])

            # ---- P9: head ---------------------------------------------------
            with (
                tc.tile_pool(name="p9sb", bufs=1) as p9sb,
                tc.tile_pool(name="p9ps", bufs=1, space="PSUM") as p9ps,
            ):
                ci = p9sb.tile([1, B], F32)
                nc.sync.dma_start(ci[:], cntinv_d[:])
                ps_ci = p9ps.tile([P, B], F32, space="PSUM", tag="ci")
                nc.tensor.matmul(ps_ci[:], lhsT=ones_row[:], rhs=ci[:],
                                 start=True, stop=True)
                cib = p9sb.tile([P, B], F32)
                nc.scalar.copy(cib[:], ps_ci[:])

                zc = []
                for ch in range(4):
                    pc = p9sb.tile([P, B], F32, tag=f"pc{ch}")
                    nc.sync.dma_start(pc[:], poolout_d[ch])
                    z = p9sb.tile([P, B], F32, tag=f"z{ch}")
                    nc.vector.tensor_mul(z[:], pc[:], cib[:])
                    zc.append(z)
                gfT = p9sb.tile([G_DIM, B], F32)
                nc.sync.dma_start(gfT[:], gfT_d[:])
                fc1 = []
                for ch in range(4):
                    w = p9sb.tile([P, C], F32, tag=f"w{ch}")
                    nc.sync.dma_start(w[:], fc1_d[ch * P:(ch + 1) * P, :])
                    fc1.append(w)
                fc1g = p9sb.tile([G_DIM, C], F32)
                nc.sync.dma_start(fc1g[:], fc1_d[HC:HC + G_DIM, :])
                fc1b = p9sb.tile([1, C], F32)
                nc.sync.dma_start(fc1b[:], fc1_d[HC + G_DIM:HC + G_DIM + 1, :])

                ps_z1 = p9ps.tile([B, C], F32, space="PSUM", tag="z1")
                for ch in range(4):
                    nc.tensor.matmul(ps_z1[:], lhsT=zc[ch][:], rhs=fc1[ch][:],
                                     start=(ch == 0), stop=False)
                nc.tensor.matmul(ps_z1[:], lhsT=gfT[:], rhs=fc1g[:],
                                 start=False, stop=False)
                nc.tensor.matmul(ps_z1[:], lhsT=ones_row[:, :B], rhs=fc1b[:],
                                 start=False, stop=True)
                z1 = p9sb.tile([B, C], F32)
                nc.scalar.activation(z1[:], ps_z1[:], AF.Relu)

                ps_z1T = p9ps.tile([C, B], F32, space="PSUM", tag="z1T")
                nc.tensor.transpose(ps_z1T[:], z1[:], ident[:B, :B])
                z1T = p9sb.tile([C, B], F32)
                nc.scalar.copy(z1T[:], ps_z1T[:])

                fc2 = p9sb.tile([C, 1], F32)
                nc.sync.dma_start(fc2[:], fc2_d[:])
                ps_o = p9ps.tile([B, 1], F32, space="PSUM", tag="o")
                nc.tensor.matmul(ps_o[:], lhsT=z1T[:], rhs=fc2[:], start=True,
                                 stop=True)
                fc2b = p9sb.tile([B, 1], F32)
                nc.sync.dma_start(fc2b[:], fc2b_d[:])
                osb = p9sb.tile([B, 1], F32)
                nc.vector.tensor_scalar_add(osb[:], ps_o[:], fc2b[:])
                nc.sync.dma_start(out_d[:], osb[:])

    nc.compile()
    return nc


def _build_null_program(t_max):
    """Same ExternalInputs as the real program, trivial compute. Used by
    bench.py to measure per-call dispatch overhead."""
    nc = bacc.Bacc("TRN2", target_bir_lowering=False, debug=False,
                   num_devices=N_CORES)
    decls = [
        ("xT_aug", [F_IN + 1, N_PAD], F32), ("xTq_aug", [F_IN + 1, NSHARD], F32),
        ("W1l_aug", [F_IN + 1, HC], F32), ("W1r_aug", [F_IN + 1, HC], F32),
        ("W2l", [HC, HC], F32), ("W2r", [HC, HC], F32),
        ("b2l_row", [1, HC], F32), ("b2r_row", [1, HC], F32),
        ("att1_bc", [P, HC], BF16), ("att2_bc", [P, HC], BF16),
        ("bn1_g_row", [1, HC], F32), ("bn1_b_row", [1, HC], F32),
        ("bias1_row", [1, HC], F32), ("bn2_g_row", [1, HC], F32),
        ("bn2_b_row", [1, HC], F32), ("bias2_row", [1, HC], F32),
        ("fc1_aug", [HC + G_DIM + 1, C], F32), ("fc2_w", [C, 1], F32),
        ("fc2_b_col", [B, 1], F32), ("gfT", [G_DIM, B], F32),
        ("cntinv_row", [1, B], F32), ("ident", [P, P], F32),
        ("identb", [P, P], BF16),
        ("ones_row", [1, P], F32), ("ones_col", [P, 1], F32),
        ("src_idx", [NBLK_CORE, P, t_max], I32),
        ("oh", [NBLK_CORE, t_max, P, P], BF16),
        ("oht", [NBLK_CORE, t_max, P, P], BF16),
        ("ohb", [NBLK_CORE, P, B], F32),
    ]
    handles = {}
    for nm, shp, dt in decls:
        handles[nm] = nc.dram_tensor(nm, shp, dt, kind="ExternalInput")
    out_d = nc.dram_tensor("out_final", [B, 1], F32, kind="ExternalOutput")
    with tile.TileContext(nc) as tc:
        with tc.tile_pool(name="sb", bufs=1) as sb:
            t = sb.tile([B, 1], F32)
            nc.sync.dma_start(t[:], handles["fc2_b_col"][:])
            nc.sync.dma_start(out_d[:], t[:])
    nc.compile()
    return nc


def _preprocess(inputs):
    """Host-side: edge sorting/sharding/padding + weight repacking."""
    x = np.asarray(inputs["x"], np.float32)
    gf = np.asarray(inputs["global_feat"], np.float32)
    ei = np.asarray(inputs["edge_index"])
    batch = np.asarray(inputs["batch"])

    src = np.concatenate([ei[0], np.arange(N)]).astype(np.int64)
    dst = np.concatenate([ei[1], np.arange(N)]).astype(np.int64)
    order = np.argsort(dst, kind="stable")
    src, dst = src[order], dst[order]
    blk = (dst // P).astype(np.int64)
    counts = np.bincount(blk, minlength=NBLK)
    t_max = max(1, int(np.ceil(counts.max() / P)))
    e_cap = t_max * P

    src_pad = np.zeros((NBLK, e_cap), np.int32)
    dstl_pad = np.full((NBLK, e_cap), 200.0, np.float32)
    starts = np.concatenate([[0], np.cumsum(counts)])
    for b in range(NBLK):
        c = counts[b]
        if c:
            s = starts[b]
            src_pad[b, :c] = src[s:s + c]
            dstl_pad[b, :c] = (dst[s:s + c] - b * P).astype(np.float32)
    # [blk, e_cap] -> [blk, 128, t_max] with edge e of tile t at [e, t]
    src_t = src_pad.reshape(NBLK, t_max, P).transpose(0, 2, 1).copy()
    # one-hots per (block, tile): OH[e, d], OHT[d, e]
    dstl_t3 = dstl_pad.reshape(NBLK, t_max, P)
    oh_all = (dstl_t3[..., None] == np.arange(P, dtype=np.float32)
              [None, None, None, :]).astype("bfloat16")     # [NBLK, t, e, d]
    oht_all = np.ascontiguousarray(oh_all.transpose(0, 1, 3, 2))

    xT_aug = np.zeros((F_IN + 1, N_PAD), np.float32)
    xT_aug[:F_IN, :N] = x.T
    xT_aug[F_IN, :] = 1.0

    def aug_w(w, bvec):
        return np.concatenate([np.asarray(w, np.float32),
                               np.asarray(bvec, np.float32)[None, :]], 0)

    att1_bc = np.tile(np.asarray(inputs["att1"], np.float32).reshape(1, HC),
                      (P, 1)).astype("bfloat16")
    att2_bc = np.tile(np.asarray(inputs["att2"], np.float32).reshape(1, HC),
                      (P, 1)).astype("bfloat16")

    fc1_aug = np.concatenate([np.asarray(inputs["fc1_w"], np.float32),
                              np.asarray(inputs["fc1_b"], np.float32)[None, :]], 0)

    cnt = np.bincount(batch.astype(np.int64), minlength=B).astype(np.float32)
    cntinv = (1.0 / np.maximum(cnt, 1.0)).reshape(1, B)

    batch_p = np.full(N_PAD, -1, np.int64)
    batch_p[:N] = batch
    ohb_all = (batch_p.reshape(NBLK, P)[:, :, None]
               == np.arange(B)[None, None, :]).astype(np.float32)

    common = {
        "xT_aug": xT_aug,
        "W1l_aug": aug_w(inputs["W1l"], inputs["b1l"]),
        "W1r_aug": aug_w(inputs["W1r"], inputs["b1r"]),
        "W2l": np.asarray(inputs["W2l"], np.float32),
        "W2r": np.asarray(inputs["W2r"], np.float32),
        "b2l_row": np.asarray(inputs["b2l"], np.float32).reshape(1, HC),
        "b2r_row": np.asarray(inputs["b2r"], np.float32).reshape(1, HC),
        "att1_bc": att1_bc,
        "att2_bc": att2_bc,
        "bn1_g_row": np.asarray(inputs["bn1_g"], np.float32).reshape(1, HC),
        "bn1_b_row": np.asarray(inputs["bn1_b"], np.float32).reshape(1, HC),
        "bias1_row": np.asarray(inputs["bias1"], np.float32).reshape(1, HC),
        "bn2_g_row": np.asarray(inputs["bn2_g"], np.float32).reshape(1, HC),
        "bn2_b_row": np.asarray(inputs["bn2_b"], np.float32).reshape(1, HC),
        "bias2_row": np.asarray(inputs["bias2"], np.float32).reshape(1, HC),
        "fc1_aug": fc1_aug,
        "fc2_w": np.asarray(inputs["fc2_w"], np.float32).reshape(C, 1),
        "fc2_b_col": np.full((B, 1), np.asarray(inputs["fc2_b"], np.float32).reshape(-1)[0], np.float32),
        "gfT": np.ascontiguousarray(gf.T),
        "cntinv_row": cntinv,
        "ident": np.eye(P, dtype=np.float32),
        "identb": np.eye(P, dtype=np.float32).astype("bfloat16"),
        "ones_row": np.ones((1, P), np.float32),
        "ones_col": np.ones((P, 1), np.float32),
    }

    in_maps = []
    for c in range(N_CORES):
        lo, hi = c * NBLK_CORE, (c + 1) * NBLK_CORE
        m = dict(common)
        m["xTq_aug"] = np.ascontiguousarray(xT_aug[:, lo * P:hi * P])
        m["src_idx"] = src_t[lo:hi]
        m["oh"] = oh_all[lo:hi]
        m["oht"] = oht_all[lo:hi]
        m["ohb"] = ohb_all[lo:hi]
        in_maps.append(m)
    return in_maps, t_max


def _run(inputs, trace=False):
    in_maps, t_max = _preprocess(inputs)
    if t_max not in _PROGRAM_CACHE:
        _PROGRAM_CACHE[t_max] = _build_program(t_max)
    nc = _PROGRAM_CACHE[t_max]
    res = run_bass_kernel_spmd(nc, in_maps, list(range(N_CORES)), trace=trace)
    out = np.asarray(res.results[0]["out_final"], np.float32).reshape(B)
    return out, res


def kernel(**inputs) -> np.ndarray:
    out, _ = _run(inputs, trace=False)
    return out



# revision 6
# speedup vs baseline: 19.2968x; 19.2968x over previous
"""GATv2WithGlobal Trainium2 kernel — 8-core SPMD bass implementation.

Strategy (dst-sharded message passing, transfer-minimized):
- Nodes padded 30000->30720, sharded as 8 cores x 30 blocks x 128 dst nodes.
- Edges (+self loops) sorted by dst, grouped per dst-block, padded to a uniform
  t_max tiles of 128 edges per block (SPMD needs one program for all cores).
- Host ships only compact data (~0.7MB/core): node-feature shard, per-edge
  source indices + local dst indices, per-node graph ids, sharded weights.
  All one-hot scatter/gather matrices, identity matrices and iota constants
  are built ON DEVICE (iota + is_equal), and layer-1 source transforms are
  computed per-shard then AllGathered — nothing large crosses the host link.
- Per edge tile: indirect-DMA gather of source features; OH[e,d]=(dst_l[e]==d)
  via vector is_equal against an iota row-values constant; OHT via PE
  transpose; one-hot matmuls for target-feature broadcast and
  scatter-aggregation with PSUM accumulation; LeakyReLU on ScalarE; attention
  scores via fused tensor_tensor_reduce; exp on ScalarE. Segment softmax
  without max-subtraction (scores are O(few), fp32-safe).
- BatchNorm via E[x^2]-mu^2 with the layer bias folded into the stats
  (variance is bias-invariant); BN applied with broadcast scale/shift.
- Cross-core exchanges: AllGather for sharded weights + source-transform
  tables, AllReduce for BN stats & pooled features.
- The PJRT executable is jitted once per program and cached; per-call work is
  host preprocessing + ~5.5MB H2D + execute.
"""

import numpy as np

import jax
from jax.sharding import Mesh, PartitionSpec
from jax.experimental.shard_map import shard_map

import concourse.bass as bass
import concourse.mybir as mybir
import concourse.tile as tile
from concourse import bacc
from concourse.bass_utils import run_bass_kernel_spmd
from concourse.bass2jax import (
    _bass_exec_p,
    install_neuronx_cc_hook,
    partition_id_tensor,
)

# problem dims (hardcoded per contract)
N = 30000
N_PAD = 30720
P = 128
N_CORES = 8
NBLK = N_PAD // P            # 240
NBLK_CORE = NBLK // N_CORES  # 30
NSHARD = NBLK_CORE * P       # 3840
H, C, HC = 4, 128, 512
F_IN, G_DIM, B = 9, 50, 64
W2_SH = HC // N_CORES        # 64 rows of W2 shipped per core
FC1_ROWS = HC + G_DIM + 1    # 563
FC1_PAD = 568                # 8 * 71
FC1_SH = FC1_PAD // N_CORES  # 71
SLOPE = 0.2
EPS_BN = 1e-5

F32 = mybir.dt.float32
BF16 = mybir.dt.bfloat16
I32 = mybir.dt.int32
AF = mybir.ActivationFunctionType
OP = mybir.AluOpType

_PROGRAM_CACHE: dict = {}


def _edge_layer(nc, pools, t_max, am_dram, xr_dram, att_bc, raw_dram,
                src_idx_d, dstl_d, consts, psBN_sum, psBN_sq):
    """One GATv2 message-passing layer over this core's 30 dst blocks (bf16).
    Writes aggregated (pre-BN, bias-free) features to raw_dram (f32) and
    accumulates BN sum/sumsq into the two persistent PSUM tiles."""
    identb, rowvals_b, ones_col = consts
    sb, psA, psT, psC, psD = pools

    for b in range(NBLK_CORE):
        idx_sb = sb.tile([P, t_max], I32, tag="idx", bufs=2)
        nc.sync.dma_start(idx_sb[:], src_idx_d[b])
        dstl_sb = sb.tile([P, t_max], F32, tag="dstl", bufs=2)
        nc.sync.dma_start(dstl_sb[:], dstl_d[b])
        xr_blk = sb.tile([P, HC], BF16, tag="xrb", bufs=2)
        nc.sync.dma_start(xr_blk[:], xr_dram[b * P:(b + 1) * P, :])

        psum_C = psC.tile([P, HC], F32, space="PSUM", tag="C")
        psum_D = psD.tile([P, H], F32, space="PSUM", tag="D")

        for t in range(t_max):
            # one-hot OH[e, d] = (dst_local[e] == d), built on device
            OH = sb.tile([P, P], BF16, tag="OH")
            nc.vector.tensor_scalar(OH[:], rowvals_b[:], dstl_sb[:, t:t + 1],
                                    None, OP.is_equal)
            ps_T = psT.tile([P, P], BF16, space="PSUM", tag="T")
            nc.tensor.transpose(ps_T[:], OH[:], identb[:])
            OHT = sb.tile([P, P], BF16, tag="OHT")
            nc.scalar.copy(OHT[:], ps_T[:])

            XL = sb.tile([P, HC], BF16, tag="XL")
            nc.gpsimd.indirect_dma_start(
                out=XL[:], out_offset=None, in_=am_dram[:],
                in_offset=bass.IndirectOffsetOnAxis(ap=idx_sb[:, t:t + 1],
                                                    axis=0),
            )

            psum_m = psA.tile([P, HC], F32, space="PSUM", tag="m")
            nc.tensor.matmul(psum_m[:], lhsT=identb[:], rhs=XL[:], start=True,
                             stop=False)
            nc.tensor.matmul(psum_m[:], lhsT=OHT[:], rhs=xr_blk[:],
                             start=False, stop=True)
            LR = sb.tile([P, HC], BF16, tag="LR")
            nc.scalar.activation(LR[:], psum_m[:], AF.Prelu, alpha=SLOPE)

            # attention scores: S[e, h] = sum_c LR[e, h, c] * att[h, c]
            TM = sb.tile([P, HC], BF16, tag="TM")
            nc.vector.tensor_mul(TM[:], LR[:], att_bc[:])
            S = sb.tile([P, H], F32, tag="S")
            nc.vector.tensor_reduce(
                out=S[:], in_=TM[:].rearrange("p (h c) -> p h c", h=H),
                axis=mybir.AxisListType.X, op=OP.add)
            P4f = sb.tile([P, H], F32, tag="P4f")
            nc.scalar.activation(P4f[:], S[:], AF.Exp)
            P4b = sb.tile([P, H], BF16, tag="P4b")
            nc.vector.tensor_copy(P4b[:], P4f[:])
            XLP = sb.tile([P, HC], BF16, tag="XLP")
            for h in range(H):
                nc.vector.tensor_scalar_mul(XLP[:, h * C:(h + 1) * C],
                                            XL[:, h * C:(h + 1) * C],
                                            P4f[:, h:h + 1])
            nc.tensor.matmul(psum_C[:], lhsT=OH[:], rhs=XLP[:], start=(t == 0),
                             stop=(t == t_max - 1))
            nc.tensor.matmul(psum_D[:], lhsT=OH[:], rhs=P4b[:], start=(t == 0),
                             stop=(t == t_max - 1))

        # block flush: OUT = C / (D + eps); BN moment accumulation
        Deps = sb.tile([P, H], F32, tag="Deps")
        nc.vector.tensor_scalar_add(Deps[:], psum_D[:], 1e-16)
        rec = sb.tile([P, H], F32, tag="rec")
        nc.vector.reciprocal(rec[:], Deps[:])
        OUT = sb.tile([P, HC], F32, tag="OUT")
        for h in range(H):
            nc.vector.tensor_scalar_mul(OUT[:, h * C:(h + 1) * C],
                                        psum_C[:, h * C:(h + 1) * C],
                                        rec[:, h:h + 1])
        nc.sync.dma_start(raw_dram[b * P:(b + 1) * P, :], OUT[:])
        SQ = sb.tile([P, HC], F32, tag="SQ")
        nc.scalar.activation(SQ[:], OUT[:], AF.Square)
        nc.tensor.matmul(psBN_sum[:], lhsT=ones_col[:], rhs=OUT[:],
                         start=(b == 0), stop=(b == NBLK_CORE - 1))
        nc.tensor.matmul(psBN_sq[:], lhsT=ones_col[:], rhs=SQ[:],
                         start=(b == 0), stop=(b == NBLK_CORE - 1))


def _bn_scale_shift(nc, hold, sb, psum_pool, stats_in_d, stats_out_d, psBN_sum,
                    psBN_sq, bng_row_d, bnb_row_d, bias_row_d, ones_row, tag,
                    collective_fn):
    """AllReduce BN moments across cores, compute broadcast scale/shift tiles.
    Small temps go in `sb` (transient pool); the returned broadcast tiles
    (scale_bc, shift_bc) [128, 512] live in `hold`."""
    stats = sb.tile([1, 2 * HC], F32, tag=f"st{tag}", bufs=1)
    nc.scalar.copy(stats[:, :HC], psBN_sum[:])
    nc.scalar.copy(stats[:, HC:], psBN_sq[:])
    nc.sync.dma_start(stats_in_d[:], stats[:])
    collective_fn("AllReduce", OP.add, [list(range(N_CORES))],
                  [stats_in_d[:]], [stats_out_d[:]])
    st = sb.tile([1, 2 * HC], F32, tag=f"str{tag}", bufs=1)
    nc.sync.dma_start(st[:], stats_out_d[:])

    bng = sb.tile([1, HC], F32, tag=f"bng{tag}", bufs=1)
    nc.sync.dma_start(bng[:], bng_row_d[:])
    bnb = sb.tile([1, HC], F32, tag=f"bnb{tag}", bufs=1)
    nc.sync.dma_start(bnb[:], bnb_row_d[:])
    bias = sb.tile([1, HC], F32, tag=f"bias{tag}", bufs=1)
    nc.sync.dma_start(bias[:], bias_row_d[:])

    inv_n = 1.0 / N
    mu0 = sb.tile([1, HC], F32, tag=f"mu0{tag}", bufs=1)
    nc.vector.tensor_scalar_mul(mu0[:], st[:, :HC], inv_n)
    ex2 = sb.tile([1, HC], F32, tag=f"ex2{tag}", bufs=1)
    nc.vector.tensor_scalar_mul(ex2[:], st[:, HC:], inv_n)
    mu0sq = sb.tile([1, HC], F32, tag=f"mu0sq{tag}", bufs=1)
    nc.vector.tensor_mul(mu0sq[:], mu0[:], mu0[:])
    var = sb.tile([1, HC], F32, tag=f"var{tag}", bufs=1)
    nc.vector.tensor_sub(var[:], ex2[:], mu0sq[:])
    vareps = sb.tile([1, HC], F32, tag=f"vareps{tag}", bufs=1)
    nc.vector.tensor_scalar_add(vareps[:], var[:], EPS_BN)
    sd = sb.tile([1, HC], F32, tag=f"sd{tag}", bufs=1)
    nc.scalar.activation(sd[:], vareps[:], AF.Sqrt)
    rsd = sb.tile([1, HC], F32, tag=f"rsd{tag}", bufs=1)
    nc.vector.reciprocal(rsd[:], sd[:])
    scale = sb.tile([1, HC], F32, tag=f"scale{tag}", bufs=1)
    nc.vector.tensor_mul(scale[:], bng[:], rsd[:])
    mup = sb.tile([1, HC], F32, tag=f"mup{tag}", bufs=1)
    nc.vector.tensor_add(mup[:], mu0[:], bias[:])
    t1 = sb.tile([1, HC], F32, tag=f"t1{tag}", bufs=1)
    nc.vector.tensor_mul(t1[:], mup[:], scale[:])
    shift = sb.tile([1, HC], F32, tag=f"shift{tag}", bufs=1)
    nc.vector.tensor_sub(shift[:], bnb[:], t1[:])

    ps_s = psum_pool.tile([P, HC], F32, space="PSUM", tag="m")
    nc.tensor.matmul(ps_s[:], lhsT=ones_row[:], rhs=scale[:], start=True,
                     stop=True)
    scale_bc = hold.tile([P, HC], F32, tag=f"scbc{tag}")
    nc.scalar.copy(scale_bc[:], ps_s[:])
    ps_h = psum_pool.tile([P, HC], F32, space="PSUM", tag="m")
    nc.tensor.matmul(ps_h[:], lhsT=ones_row[:], rhs=shift[:], start=True,
                     stop=True)
    shift_bc = hold.tile([P, HC], F32, tag=f"shbc{tag}")
    nc.scalar.copy(shift_bc[:], ps_h[:])
    return scale_bc, shift_bc


def _build_program(t_max):
    nc = bacc.Bacc("TRN2", target_bir_lowering=False, debug=False,
                   num_devices=N_CORES)

    def _collective(kind, op, groups_, ins, outs):
        nc.gpsimd.collective_compute(kind, op, replica_groups=groups_,
                                     ins=ins, outs=outs)

    # ---- I/O declarations -------------------------------------------------
    xTq_d = nc.dram_tensor("xTq_aug", [F_IN + 1, NSHARD], F32, kind="ExternalInput")
    W1l_d = nc.dram_tensor("W1l_aug", [F_IN + 1, HC], F32, kind="ExternalInput")
    W1r_d = nc.dram_tensor("W1r_aug", [F_IN + 1, HC], F32, kind="ExternalInput")
    W2ls_d = nc.dram_tensor("W2l_shard", [W2_SH, HC], F32, kind="ExternalInput")
    W2rs_d = nc.dram_tensor("W2r_shard", [W2_SH, HC], F32, kind="ExternalInput")
    b2l_d = nc.dram_tensor("b2l_row", [1, HC], F32, kind="ExternalInput")
    b2r_d = nc.dram_tensor("b2r_row", [1, HC], F32, kind="ExternalInput")
    att1_d = nc.dram_tensor("att1_row", [1, HC], BF16, kind="ExternalInput")
    att2_d = nc.dram_tensor("att2_row", [1, HC], BF16, kind="ExternalInput")
    bn1g_d = nc.dram_tensor("bn1_g_row", [1, HC], F32, kind="ExternalInput")
    bn1b_d = nc.dram_tensor("bn1_b_row", [1, HC], F32, kind="ExternalInput")
    bias1_d = nc.dram_tensor("bias1_row", [1, HC], F32, kind="ExternalInput")
    bn2g_d = nc.dram_tensor("bn2_g_row", [1, HC], F32, kind="ExternalInput")
    bn2b_d = nc.dram_tensor("bn2_b_row", [1, HC], F32, kind="ExternalInput")
    bias2_d = nc.dram_tensor("bias2_row", [1, HC], F32, kind="ExternalInput")
    fc1s_d = nc.dram_tensor("fc1_shard", [FC1_SH, C], F32, kind="ExternalInput")
    fc2_d = nc.dram_tensor("fc2_w", [C, 1], F32, kind="ExternalInput")
    fc2b_d = nc.dram_tensor("fc2_b_col", [B, 1], F32, kind="ExternalInput")
    gfT_d = nc.dram_tensor("gfT", [G_DIM, B], F32, kind="ExternalInput")
    cntinv_d = nc.dram_tensor("cntinv_row", [1, B], F32, kind="ExternalInput")
    src_idx_d = nc.dram_tensor("src_idx", [NBLK_CORE, P, t_max], I32, kind="ExternalInput")
    dstl_d = nc.dram_tensor("dstl", [NBLK_CORE, P, t_max], F32, kind="ExternalInput")
    batch_d = nc.dram_tensor("batch_col", [NBLK_CORE, P, 1], F32, kind="ExternalInput")

    out_d = nc.dram_tensor("out_final", [B, 1], F32, kind="ExternalOutput")

    # internal DRAM
    am1s_d = nc.dram_tensor("am1s", [NSHARD, HC], BF16)
    am1_d = nc.dram_tensor("am1", [N_PAD, HC], BF16, addr_space="Shared")
    xr1_d = nc.dram_tensor("xr1", [NSHARD, HC], BF16)
    xr2_d = nc.dram_tensor("xr2", [NSHARD, HC], BF16)
    hT_d = nc.dram_tensor("hT", [HC, NSHARD], F32)
    h1raw_d = nc.dram_tensor("h1raw", [NSHARD, HC], F32)
    am2s_d = nc.dram_tensor("am2s", [NSHARD, HC], BF16)
    am2_d = nc.dram_tensor("am2", [N_PAD, HC], BF16, addr_space="Shared")
    h2raw_d = nc.dram_tensor("h2raw", [NSHARD, HC], F32)
    W2l_full_d = nc.dram_tensor("W2l_full", [HC, HC], F32, addr_space="Shared")
    W2r_full_d = nc.dram_tensor("W2r_full", [HC, HC], F32, addr_space="Shared")
    fc1_full_d = nc.dram_tensor("fc1_full", [FC1_PAD, C], F32, addr_space="Shared")
    bn1in_d = nc.dram_tensor("bn1in", [1, 2 * HC], F32)
    bn1out_d = nc.dram_tensor("bn1out", [1, 2 * HC], F32, addr_space="Shared")
    bn2in_d = nc.dram_tensor("bn2in", [1, 2 * HC], F32)
    bn2out_d = nc.dram_tensor("bn2out", [1, 2 * HC], F32, addr_space="Shared")
    poolin_d = nc.dram_tensor("poolin", [H, P, B], F32)
    poolout_d = nc.dram_tensor("poolout", [H, P, B], F32, addr_space="Shared")
    W2l_in_d = nc.dram_tensor("W2l_in", [W2_SH, HC], F32)
    W2r_in_d = nc.dram_tensor("W2r_in", [W2_SH, HC], F32)
    fc1_in_d = nc.dram_tensor("fc1_in", [FC1_SH, C], F32)

    groups = [list(range(N_CORES))]

    with tile.TileContext(nc) as tc:
        # sharded-weight AllGathers: no deps, overlap with early compute
        # (collectives may not read IO tensors -> stage via internal DRAM)
        nc.sync.dma_start(W2l_in_d[:], W2ls_d[:])
        nc.sync.dma_start(W2r_in_d[:], W2rs_d[:])
        nc.sync.dma_start(fc1_in_d[:], fc1s_d[:])
        _collective("AllGather", OP.bypass, groups, [W2l_in_d[:]], [W2l_full_d[:]])
        _collective("AllGather", OP.bypass, groups, [W2r_in_d[:]], [W2r_full_d[:]])
        _collective("AllGather", OP.bypass, groups, [fc1_in_d[:]], [fc1_full_d[:]])

        with (
            tc.tile_pool(name="const", bufs=1) as cpool,
            tc.tile_pool(name="hold", bufs=1) as hold,
        ):
            # ---- P0: constants built on device --------------------------
            rowvals_b = cpool.tile([P, P], BF16)      # [p, f] = f
            nc.gpsimd.iota(rowvals_b[:], [[1, P]], channel_multiplier=0,
                           allow_small_or_imprecise_dtypes=True)
            rowvals_f = cpool.tile([P, P], F32)
            nc.gpsimd.iota(rowvals_f[:], [[1, P]], channel_multiplier=0,
                           allow_small_or_imprecise_dtypes=True)
            iota_col_f = cpool.tile([P, 1], F32)      # [p, 0] = p
            nc.gpsimd.iota(iota_col_f[:], [[0, 1]], channel_multiplier=1,
                           allow_small_or_imprecise_dtypes=True)
            ident = cpool.tile([P, P], F32)
            nc.vector.tensor_scalar(ident[:], rowvals_f[:], iota_col_f[:],
                                    None, OP.is_equal)
            identb = cpool.tile([P, P], BF16)
            nc.vector.tensor_scalar(identb[:], rowvals_b[:], iota_col_f[:],
                                    None, OP.is_equal)
            ones_row = cpool.tile([1, P], F32)
            nc.vector.memset(ones_row[:], 1.0)
            ones_rowb = cpool.tile([1, P], BF16)
            nc.vector.memset(ones_rowb[:], 1.0)
            ones_col = cpool.tile([P, 1], F32)
            nc.vector.memset(ones_col[:], 1.0)

            edge_consts = (identb, rowvals_b, ones_col)

            # att rows -> [P, HC] broadcast tiles (outer product with ones)
            with tc.tile_pool(name="p0ps", bufs=2, space="PSUM") as p0ps:
                att1_row = cpool.tile([1, HC], BF16)
                nc.sync.dma_start(att1_row[:], att1_d[:])
                att2_row = cpool.tile([1, HC], BF16)
                nc.sync.dma_start(att2_row[:], att2_d[:])
                ps_a1 = p0ps.tile([P, HC], F32, space="PSUM", tag="a")
                nc.tensor.matmul(ps_a1[:], lhsT=ones_rowb[:], rhs=att1_row[:],
                                 start=True, stop=True)
                att1_bc = cpool.tile([P, HC], BF16)
                nc.scalar.copy(att1_bc[:], ps_a1[:])
                ps_a2 = p0ps.tile([P, HC], F32, space="PSUM", tag="a")
                nc.tensor.matmul(ps_a2[:], lhsT=ones_rowb[:], rhs=att2_row[:],
                                 start=True, stop=True)
                att2_bc = cpool.tile([P, HC], BF16)
                nc.scalar.copy(att2_bc[:], ps_a2[:])

            # ---- P1: layer-1 node transforms (own shard only) -----------
            with (
                tc.tile_pool(name="p1sb", bufs=3) as p1sb,
                tc.tile_pool(name="p1ps", bufs=4, space="PSUM") as p1ps,
            ):
                W1l = p1sb.tile([F_IN + 1, HC], F32, bufs=1)
                nc.sync.dma_start(W1l[:], W1l_d[:])
                W1r = p1sb.tile([F_IN + 1, HC], F32, bufs=1)
                nc.sync.dma_start(W1r[:], W1r_d[:])
                for j in range(NBLK_CORE):
                    xtq = p1sb.tile([F_IN + 1, P], F32, tag="xtq")
                    nc.sync.dma_start(xtq[:], xTq_d[:, j * P:(j + 1) * P])
                    ps = p1ps.tile([P, HC], F32, space="PSUM", tag="p1")
                    nc.tensor.matmul(ps[:], lhsT=xtq[:], rhs=W1l[:],
                                     start=True, stop=True)
                    ev = p1sb.tile([P, HC], BF16, tag="ev")
                    nc.scalar.copy(ev[:], ps[:])
                    nc.sync.dma_start(am1s_d[j * P:(j + 1) * P, :], ev[:])
                    ps2 = p1ps.tile([P, HC], F32, space="PSUM", tag="p1")
                    nc.tensor.matmul(ps2[:], lhsT=xtq[:], rhs=W1r[:],
                                     start=True, stop=True)
                    ev2 = p1sb.tile([P, HC], BF16, tag="ev2")
                    nc.scalar.copy(ev2[:], ps2[:])
                    nc.sync.dma_start(xr1_d[j * P:(j + 1) * P, :], ev2[:])

            # gather the full layer-1 source-transform table
            _collective("AllGather", OP.bypass, groups, [am1s_d[:]], [am1_d[:]])

            # ---- P2: layer-1 edge aggregation ---------------------------
            with (
                tc.tile_pool(name="e1sb", bufs=6) as esb,
                tc.tile_pool(name="e1psA", bufs=3, space="PSUM") as psA,
                tc.tile_pool(name="e1psT", bufs=1, space="PSUM") as psT,
                tc.tile_pool(name="e1psC", bufs=1, space="PSUM") as psC,
                tc.tile_pool(name="e1psD", bufs=1, space="PSUM") as psD,
                tc.tile_pool(name="e1psBN", bufs=1, space="PSUM") as psBN,
            ):
                psBN_sum = psBN.tile([1, HC], F32, space="PSUM")
                psBN_sq = psBN.tile([1, HC], F32, space="PSUM")
                _edge_layer(nc, (esb, psA, psT, psC, psD), t_max, am1_d,
                            xr1_d, att1_bc, h1raw_d, src_idx_d, dstl_d,
                            edge_consts, psBN_sum, psBN_sq)

                # ---- P3: BN1 stats + scale/shift ------------------------
                scale1_bc, shift1_bc = _bn_scale_shift(
                    nc, hold, esb, psA, bn1in_d, bn1out_d, psBN_sum, psBN_sq,
                    bn1g_d, bn1b_d, bias1_d, ones_row, "b1", _collective)

            # ---- P4: BN1 apply + relu + build hT ------------------------
            with (
                tc.tile_pool(name="p4sb", bufs=3) as p4sb,
                tc.tile_pool(name="p4ps", bufs=2, space="PSUM") as p4ps,
            ):
                for j in range(NBLK_CORE):
                    raw = p4sb.tile([P, HC], F32, tag="raw")
                    nc.sync.dma_start(raw[:], h1raw_d[j * P:(j + 1) * P, :])
                    t1 = p4sb.tile([P, HC], F32, tag="t1")
                    nc.vector.tensor_mul(t1[:], raw[:], scale1_bc[:])
                    t2 = p4sb.tile([P, HC], F32, tag="t2")
                    nc.vector.tensor_add(t2[:], t1[:], shift1_bc[:])
                    hsb = p4sb.tile([P, HC], F32, tag="h")
                    nc.vector.tensor_scalar_max(hsb[:], t2[:], 0.0)
                    pst = p4ps.tile([P, HC], F32, space="PSUM", tag="tr")
                    for ch in range(4):
                        nc.tensor.transpose(pst[:, ch * P:(ch + 1) * P],
                                            hsb[:, ch * P:(ch + 1) * P],
                                            ident[:])
                    ev4 = p4sb.tile([P, HC], F32, tag="ev4")
                    nc.scalar.copy(ev4[:], pst[:])
                    for ch in range(4):
                        nc.sync.dma_start(
                            hT_d[ch * P:(ch + 1) * P, j * P:(j + 1) * P],
                            ev4[:, ch * P:(ch + 1) * P])

            # ---- P5: layer-2 node transforms ----------------------------
            with (
                tc.tile_pool(name="p5sb", bufs=3) as p5sb,
                tc.tile_pool(name="p5w", bufs=1) as p5w,
                tc.tile_pool(name="p5ps", bufs=4, space="PSUM") as p5ps,
            ):
                W2l_sb = [p5w.tile([P, HC], F32, name=f"W2l{k}", tag=f"W2l{k}")
                          for k in range(4)]
                W2r_sb = [p5w.tile([P, HC], F32, name=f"W2r{k}", tag=f"W2r{k}")
                          for k in range(4)]
                for k in range(4):
                    nc.sync.dma_start(W2l_sb[k][:],
                                      W2l_full_d[k * P:(k + 1) * P, :])
                    nc.sync.dma_start(W2r_sb[k][:],
                                      W2r_full_d[k * P:(k + 1) * P, :])
                b2l = p5w.tile([1, HC], F32)
                nc.sync.dma_start(b2l[:], b2l_d[:])
                b2r = p5w.tile([1, HC], F32)
                nc.sync.dma_start(b2r[:], b2r_d[:])
                for j in range(NBLK_CORE):
                    hTj = []
                    for k in range(4):
                        hx = p5sb.tile([P, P], F32, tag=f"hTj{k}",
                                       name=f"hTj{k}")
                        nc.sync.dma_start(
                            hx[:], hT_d[k * P:(k + 1) * P, j * P:(j + 1) * P])
                        hTj.append(hx)
                    psl = p5ps.tile([P, HC], F32, space="PSUM", tag="l")
                    for k in range(4):
                        nc.tensor.matmul(psl[:], lhsT=hTj[k][:],
                                         rhs=W2l_sb[k][:], start=(k == 0),
                                         stop=False)
                    nc.tensor.matmul(psl[:], lhsT=ones_row[:], rhs=b2l[:],
                                     start=False, stop=True)
                    ev = p5sb.tile([P, HC], BF16, tag="ev")
                    nc.scalar.copy(ev[:], psl[:])
                    nc.sync.dma_start(am2s_d[j * P:(j + 1) * P, :], ev[:])
                    psr = p5ps.tile([P, HC], F32, space="PSUM", tag="r")
                    for k in range(4):
                        nc.tensor.matmul(psr[:], lhsT=hTj[k][:],
                                         rhs=W2r_sb[k][:], start=(k == 0),
                                         stop=False)
                    nc.tensor.matmul(psr[:], lhsT=ones_row[:], rhs=b2r[:],
                                     start=False, stop=True)
                    ev5 = p5sb.tile([P, HC], BF16, tag="ev5")
                    nc.scalar.copy(ev5[:], psr[:])
                    nc.sync.dma_start(xr2_d[j * P:(j + 1) * P, :], ev5[:])

            # ---- P6: AllGather layer-2 source transforms ----------------
            _collective("AllGather", OP.bypass, groups, [am2s_d[:]], [am2_d[:]])

            # ---- P7: layer-2 edge aggregation ---------------------------
            with (
                tc.tile_pool(name="e2sb", bufs=6) as esb,
                tc.tile_pool(name="e2psA", bufs=3, space="PSUM") as psA,
                tc.tile_pool(name="e2psT", bufs=1, space="PSUM") as psT,
                tc.tile_pool(name="e2psC", bufs=1, space="PSUM") as psC,
                tc.tile_pool(name="e2psD", bufs=1, space="PSUM") as psD,
                tc.tile_pool(name="e2psBN", bufs=1, space="PSUM") as psBN,
            ):
                psBN_sum = psBN.tile([1, HC], F32, space="PSUM")
                psBN_sq = psBN.tile([1, HC], F32, space="PSUM")
                _edge_layer(nc, (esb, psA, psT, psC, psD), t_max, am2_d,
                            xr2_d, att2_bc, h2raw_d, src_idx_d, dstl_d,
                            edge_consts, psBN_sum, psBN_sq)
                scale2_bc, shift2_bc = _bn_scale_shift(
                    nc, hold, esb, psA, bn2in_d, bn2out_d, psBN_sum, psBN_sq,
                    bn2g_d, bn2b_d, bias2_d, ones_row, "b2", _collective)

            # ---- P8: BN2 apply + relu + pooling -------------------------
            with (
                tc.tile_pool(name="p8sb", bufs=3) as p8sb,
                tc.tile_pool(name="p8ps", bufs=1, space="PSUM") as p8ps,
            ):
                pool_ps = [p8ps.tile([P, B], F32, space="PSUM",
                                     name=f"pool{k}", tag=f"pool{k}")
                           for k in range(4)]
                for j in range(NBLK_CORE):
                    raw = p8sb.tile([P, HC], F32, tag="raw")
                    nc.sync.dma_start(raw[:], h2raw_d[j * P:(j + 1) * P, :])
                    t1 = p8sb.tile([P, HC], F32, tag="t1")
                    nc.vector.tensor_mul(t1[:], raw[:], scale2_bc[:])
                    t2 = p8sb.tile([P, HC], F32, tag="t2")
                    nc.vector.tensor_add(t2[:], t1[:], shift2_bc[:])
                    hsb = p8sb.tile([P, HC], F32, tag="h")
                    nc.vector.tensor_scalar_max(hsb[:], t2[:], 0.0)
                    batch_sb = p8sb.tile([P, 1], F32, tag="bat")
                    nc.sync.dma_start(batch_sb[:], batch_d[j])
                    ohb = p8sb.tile([P, B], F32, tag="ohb")
                    nc.vector.tensor_scalar(ohb[:], rowvals_f[:, :B],
                                            batch_sb[:], None, OP.is_equal)
                    for ch in range(4):
                        nc.tensor.matmul(pool_ps[ch][:],
                                         lhsT=hsb[:, ch * P:(ch + 1) * P],
                                         rhs=ohb[:], start=(j == 0),
                                         stop=(j == NBLK_CORE - 1))
                poolsb = p8sb.tile([P, 4 * B], F32)
                for ch in range(4):
                    nc.scalar.copy(poolsb[:, ch * B:(ch + 1) * B],
                                   pool_ps[ch][:])
                for ch in range(4):
                    nc.sync.dma_start(poolin_d[ch],
                                      poolsb[:, ch * B:(ch + 1) * B])
                _collective("AllReduce", OP.add, groups,
                            [poolin_d[:]], [poolout_d[:]])

            # ---- P9: head -----------------------------------------------
            with (
                tc.tile_pool(name="p9sb", bufs=1) as p9sb,
                tc.tile_pool(name="p9ps", bufs=1, space="PSUM") as p9ps,
            ):
                ci = p9sb.tile([1, B], F32)
                nc.sync.dma_start(ci[:], cntinv_d[:])
                ps_ci = p9ps.tile([P, B], F32, space="PSUM", tag="ci")
                nc.tensor.matmul(ps_ci[:], lhsT=ones_row[:], rhs=ci[:],
                                 start=True, stop=True)
                cib = p9sb.tile([P, B], F32)
                nc.scalar.copy(cib[:], ps_ci[:])

                zc = []
                for ch in range(4):
                    pc = p9sb.tile([P, B], F32, tag=f"pc{ch}")
                    nc.sync.dma_start(pc[:], poolout_d[ch])
                    z = p9sb.tile([P, B], F32, tag=f"z{ch}")
                    nc.vector.tensor_mul(z[:], pc[:], cib[:])
                    zc.append(z)
                gfT = p9sb.tile([G_DIM, B], F32)
                nc.sync.dma_start(gfT[:], gfT_d[:])
                fc1 = []
                for ch in range(4):
                    w = p9sb.tile([P, C], F32, tag=f"w{ch}")
                    nc.sync.dma_start(w[:], fc1_full_d[ch * P:(ch + 1) * P, :])
                    fc1.append(w)
                fc1g = p9sb.tile([G_DIM, C], F32)
                nc.sync.dma_start(fc1g[:], fc1_full_d[HC:HC + G_DIM, :])
                fc1b = p9sb.tile([1, C], F32)
                nc.sync.dma_start(fc1b[:],
                                  fc1_full_d[HC + G_DIM:HC + G_DIM + 1, :])

                ps_z1 = p9ps.tile([B, C], F32, space="PSUM", tag="z1")
                for ch in range(4):
                    nc.tensor.matmul(ps_z1[:], lhsT=zc[ch][:], rhs=fc1[ch][:],
                                     start=(ch == 0), stop=False)
                nc.tensor.matmul(ps_z1[:], lhsT=gfT[:], rhs=fc1g[:],
                                 start=False, stop=False)
                nc.tensor.matmul(ps_z1[:], lhsT=ones_row[:, :B], rhs=fc1b[:],
                                 start=False, stop=True)
                z1 = p9sb.tile([B, C], F32)
                nc.scalar.activation(z1[:], ps_z1[:], AF.Relu)

                ps_z1T = p9ps.tile([C, B], F32, space="PSUM", tag="z1T")
                nc.tensor.transpose(ps_z1T[:], z1[:], ident[:B, :B])
                z1T = p9sb.tile([C, B], F32)
                nc.scalar.copy(z1T[:], ps_z1T[:])

                fc2 = p9sb.tile([C, 1], F32)
                nc.sync.dma_start(fc2[:], fc2_d[:])
                ps_o = p9ps.tile([B, 1], F32, space="PSUM", tag="o")
                nc.tensor.matmul(ps_o[:], lhsT=z1T[:], rhs=fc2[:], start=True,
                                 stop=True)
                fc2b = p9sb.tile([B, 1], F32)
                nc.sync.dma_start(fc2b[:], fc2b_d[:])
                osb = p9sb.tile([B, 1], F32)
                nc.vector.tensor_scalar_add(osb[:], ps_o[:], fc2b[:])
                nc.sync.dma_start(out_d[:], osb[:])

    nc.compile()
    return nc


def _make_runner(nc):
    """Jit the PJRT executable once; returns run(in_maps) -> out_final[B]."""
    install_neuronx_cc_hook()
    partition_name = (nc.partition_id_tensor.name
                      if nc.partition_id_tensor else None)
    in_names, out_names, out_avals, zero_outs = [], [], [], []
    for alloc in nc.m.functions[0].allocations:
        if not isinstance(alloc, mybir.MemoryLocationSet):
            continue
        name = alloc.memorylocations[0].name
        if alloc.kind == "ExternalInput":
            if name != partition_name:
                in_names.append(name)
        elif alloc.kind == "ExternalOutput":
            out_names.append(name)
            shape = tuple(alloc.tensor_shape)
            dtype = mybir.dt.np(alloc.dtype)
            out_avals.append(jax.core.ShapedArray(shape, dtype))
            zero_outs.append(np.zeros(shape, dtype))
    n_params = len(in_names)
    n_outs = len(out_avals)
    in_names.extend(out_names)
    if partition_name is not None:
        in_names.append(partition_name)
    donate = tuple(range(n_params, n_params + n_outs))

    def _body(*args):
        operands = list(args)
        if partition_name is not None:
            operands.append(partition_id_tensor())
        return tuple(_bass_exec_p.bind(
            *operands, out_avals=tuple(out_avals), in_names=tuple(in_names),
            out_names=tuple(out_names), lowering_input_output_aliases=(),
            sim_require_finite=True, sim_require_nnan=True, nc=nc))

    devices = jax.devices()[:N_CORES]
    mesh = Mesh(np.asarray(devices), ("core",))
    in_specs = (PartitionSpec("core"),) * (n_params + n_outs)
    out_specs = (PartitionSpec("core"),) * len(out_names)
    sharded = jax.jit(
        shard_map(_body, mesh=mesh, in_specs=in_specs, out_specs=out_specs,
                  check_rep=False),
        donate_argnums=donate, keep_unused=True)
    param_names = in_names[:n_params]

    def run(in_maps):
        concat_in = [
            np.concatenate([np.asarray(m[name]) for m in in_maps], axis=0)
            for name in param_names
        ]
        concat_zeros = [
            np.zeros((N_CORES * z.shape[0], *z.shape[1:]), z.dtype)
            for z in zero_outs
        ]
        out_arrs = sharded(*concat_in, *concat_zeros)
        oi = out_names.index("out_final")
        return np.asarray(out_arrs[oi])[:B].reshape(B)

    return run


def _preprocess(inputs):
    """Host-side: edge sorting/sharding/padding + weight repacking."""
    x = np.asarray(inputs["x"], np.float32)
    gf = np.asarray(inputs["global_feat"], np.float32)
    ei = np.asarray(inputs["edge_index"])
    batch = np.asarray(inputs["batch"]).astype(np.int64)

    loops = np.arange(N, dtype=np.int64)
    src = np.concatenate([ei[0].astype(np.int64), loops])
    dst = np.concatenate([ei[1].astype(np.int64), loops])
    order = np.argsort(dst, kind="stable")
    src, dst = src[order], dst[order]
    blk = dst >> 7
    counts = np.bincount(blk, minlength=NBLK)
    t_max = max(1, int(np.ceil(counts.max() / P)))
    e_cap = t_max * P

    starts = np.concatenate([[0], np.cumsum(counts)])
    pos = np.arange(dst.shape[0]) - starts[blk]
    src_pad = np.zeros((NBLK, e_cap), np.int32)
    src_pad[blk, pos] = src
    dstl_pad = np.full((NBLK, e_cap), 200.0, np.float32)
    dstl_pad[blk, pos] = (dst - (blk << 7)).astype(np.float32)
    # [blk, e_cap] -> [blk, 128, t_max] with edge e of tile t at [e, t]
    src_t = np.ascontiguousarray(
        src_pad.reshape(NBLK, t_max, P).transpose(0, 2, 1))
    dstl_t = np.ascontiguousarray(
        dstl_pad.reshape(NBLK, t_max, P).transpose(0, 2, 1))

    xT_aug = np.zeros((F_IN + 1, N_PAD), np.float32)
    xT_aug[:F_IN, :N] = x.T
    xT_aug[F_IN, :] = 1.0

    def aug_w(w, bvec):
        return np.concatenate([np.asarray(w, np.float32),
                               np.asarray(bvec, np.float32)[None, :]], 0)

    W2l = np.ascontiguousarray(np.asarray(inputs["W2l"], np.float32))
    W2r = np.ascontiguousarray(np.asarray(inputs["W2r"], np.float32))

    fc1_pad = np.zeros((FC1_PAD, C), np.float32)
    fc1_pad[:HC] = np.asarray(inputs["fc1_w"], np.float32)[:HC]
    fc1_pad[HC:HC + G_DIM] = np.asarray(inputs["fc1_w"], np.float32)[HC:]
    fc1_pad[HC + G_DIM] = np.asarray(inputs["fc1_b"], np.float32)

    cnt = np.bincount(batch, minlength=B).astype(np.float32)
    cntinv = (1.0 / np.maximum(cnt, 1.0)).reshape(1, B)

    batch_p = np.full(N_PAD, -1.0, np.float32)
    batch_p[:N] = batch.astype(np.float32)
    batch_col = batch_p.reshape(NBLK, P, 1)

    common = {
        "W1l_aug": aug_w(inputs["W1l"], inputs["b1l"]),
        "W1r_aug": aug_w(inputs["W1r"], inputs["b1r"]),
        "b2l_row": np.asarray(inputs["b2l"], np.float32).reshape(1, HC),
        "b2r_row": np.asarray(inputs["b2r"], np.float32).reshape(1, HC),
        "att1_row": np.asarray(inputs["att1"], np.float32).reshape(1, HC).astype("bfloat16"),
        "att2_row": np.asarray(inputs["att2"], np.float32).reshape(1, HC).astype("bfloat16"),
        "bn1_g_row": np.asarray(inputs["bn1_g"], np.float32).reshape(1, HC),
        "bn1_b_row": np.asarray(inputs["bn1_b"], np.float32).reshape(1, HC),
        "bias1_row": np.asarray(inputs["bias1"], np.float32).reshape(1, HC),
        "bn2_g_row": np.asarray(inputs["bn2_g"], np.float32).reshape(1, HC),
        "bn2_b_row": np.asarray(inputs["bn2_b"], np.float32).reshape(1, HC),
        "bias2_row": np.asarray(inputs["bias2"], np.float32).reshape(1, HC),
        "fc2_w": np.asarray(inputs["fc2_w"], np.float32).reshape(C, 1),
        "fc2_b_col": np.full((B, 1), np.asarray(inputs["fc2_b"], np.float32).reshape(-1)[0], np.float32),
        "gfT": np.ascontiguousarray(gf.T),
        "cntinv_row": cntinv,
    }

    in_maps = []
    for c in range(N_CORES):
        lo, hi = c * NBLK_CORE, (c + 1) * NBLK_CORE
        m = dict(common)
        m["xTq_aug"] = np.ascontiguousarray(xT_aug[:, lo * P:hi * P])
        m["W2l_shard"] = W2l[c * W2_SH:(c + 1) * W2_SH]
        m["W2r_shard"] = W2r[c * W2_SH:(c + 1) * W2_SH]
        m["fc1_shard"] = fc1_pad[c * FC1_SH:(c + 1) * FC1_SH]
        m["src_idx"] = src_t[lo:hi]
        m["dstl"] = dstl_t[lo:hi]
        m["batch_col"] = batch_col[lo:hi]
        in_maps.append(m)
    return in_maps, t_max


class _FastResult:
    exec_time_ns = None


def _run(inputs, trace=False):
    in_maps, t_max = _preprocess(inputs)
    if t_max not in _PROGRAM_CACHE:
        nc = _build_program(t_max)
        runner = _make_runner(nc)
        _PROGRAM_CACHE[t_max] = (nc, runner)
    nc, runner = _PROGRAM_CACHE[t_max]
    if trace:
        res = run_bass_kernel_spmd(nc, in_maps, list(range(N_CORES)),
                                   trace=True)
        out = np.asarray(res.results[0]["out_final"], np.float32).reshape(B)
        return out, res
    out = runner(in_maps)
    return np.asarray(out, np.float32), _FastResult()


def kernel(**inputs) -> np.ndarray:
    out, _ = _run(inputs, trace=False)
    return out


# revision 9
# speedup vs baseline: 31.3137x; 1.6227x over previous
"""GATv2WithGlobal Trainium2 kernel — 8-core SPMD bass implementation.

Strategy (dst-sharded message passing, transfer-minimized):
- Nodes padded 30000->30720, sharded as 8 cores x 30 blocks x 128 dst nodes.
- Edges (+self loops) sorted by dst, grouped per dst-block, padded to a uniform
  t_max tiles of 128 edges per block (SPMD needs one program for all cores).
- Host ships only compact data (~0.7MB/core): node-feature shard, per-edge
  source indices + local dst indices, per-node graph ids, sharded weights.
  All one-hot scatter/gather matrices, identity matrices and iota constants
  are built ON DEVICE (iota + is_equal), and layer-1 source transforms are
  computed per-shard then AllGathered — nothing large crosses the host link.
- Per edge tile: indirect-DMA gather of source features; OH[e,d]=(dst_l[e]==d)
  via vector is_equal against an iota row-values constant; OHT via PE
  transpose; one-hot matmuls for target-feature broadcast and
  scatter-aggregation with PSUM accumulation; LeakyReLU on ScalarE; attention
  scores via fused tensor_tensor_reduce; exp on ScalarE. Segment softmax
  without max-subtraction (scores are O(few), fp32-safe).
- BatchNorm via E[x^2]-mu^2 with the layer bias folded into the stats
  (variance is bias-invariant); BN applied with broadcast scale/shift.
- Cross-core exchanges: AllGather for sharded weights + source-transform
  tables, AllReduce for BN stats & pooled features.
- The PJRT executable is jitted once per program and cached; per-call work is
  host preprocessing + ~5.5MB H2D + execute.
"""

import numpy as np

import jax
from jax.sharding import Mesh, PartitionSpec
from jax.experimental.shard_map import shard_map

import concourse.bass as bass
import concourse.mybir as mybir
import concourse.tile as tile
from concourse import bacc
from concourse.bass_utils import run_bass_kernel_spmd
from concourse.bass2jax import (
    _bass_exec_p,
    install_neuronx_cc_hook,
    partition_id_tensor,
)

# problem dims (hardcoded per contract)
N = 30000
N_PAD = 30720
P = 128
N_CORES = 8
NBLK = N_PAD // P            # 240
NBLK_CORE = NBLK // N_CORES  # 30
NSHARD = NBLK_CORE * P       # 3840
H, C, HC = 4, 128, 512
F_IN, G_DIM, B = 9, 50, 64
W2_SH = HC // N_CORES        # 64 rows of W2 shipped per core
FC1_ROWS = HC + G_DIM + 1    # 563
FC1_PAD = 568                # 8 * 71
FC1_SH = FC1_PAD // N_CORES  # 71
SLOPE = 0.2
EPS_BN = 1e-5

F32 = mybir.dt.float32
BF16 = mybir.dt.bfloat16
I32 = mybir.dt.int32
I16 = mybir.dt.int16
U8 = mybir.dt.uint8
AF = mybir.ActivationFunctionType
OP = mybir.AluOpType

_PROGRAM_CACHE: dict = {}


def _edge_layer(nc, pools, t_max, am_dram, xr_dram, att_bc, raw_dram,
                src_idx_d, dstl_d, consts, psBN_sum, psBN_sq):
    """One GATv2 message-passing layer over this core's 30 dst blocks (bf16).
    Writes aggregated (pre-BN, bias-free) features to raw_dram (f32) and
    accumulates BN sum/sumsq into the two persistent PSUM tiles."""
    identb, rowvals_b, ones_col = consts
    sb, psA, psT, psC, psD = pools

    for b in range(NBLK_CORE):
        s16 = sb.tile([P, t_max], I16, tag="s16", bufs=2)
        nc.sync.dma_start(s16[:], src_idx_d[b])
        idx_sb = sb.tile([P, t_max], I32, tag="idx", bufs=2)
        nc.vector.tensor_copy(idx_sb[:], s16[:])
        d8 = sb.tile([P, t_max], U8, tag="d8", bufs=2)
        nc.sync.dma_start(d8[:], dstl_d[b])
        dstl_sb = sb.tile([P, t_max], F32, tag="dstl", bufs=2)
        nc.vector.tensor_copy(dstl_sb[:], d8[:])
        xr_blk = sb.tile([P, HC], BF16, tag="xrb", bufs=2)
        nc.sync.dma_start(xr_blk[:], xr_dram[b * P:(b + 1) * P, :])

        psum_C = psC.tile([P, HC], F32, space="PSUM", tag="C")
        psum_D = psD.tile([P, H], F32, space="PSUM", tag="D")

        for t in range(t_max):
            # one-hot OH[e, d] = (dst_local[e] == d), built on device
            OH = sb.tile([P, P], BF16, tag="OH")
            nc.vector.tensor_scalar(OH[:], rowvals_b[:], dstl_sb[:, t:t + 1],
                                    None, OP.is_equal)
            ps_T = psT.tile([P, P], BF16, space="PSUM", tag="T")
            nc.tensor.transpose(ps_T[:], OH[:], identb[:])
            OHT = sb.tile([P, P], BF16, tag="OHT")
            nc.scalar.copy(OHT[:], ps_T[:])

            XL = sb.tile([P, HC], BF16, tag="XL")
            nc.gpsimd.indirect_dma_start(
                out=XL[:], out_offset=None, in_=am_dram[:],
                in_offset=bass.IndirectOffsetOnAxis(ap=idx_sb[:, t:t + 1],
                                                    axis=0),
            )

            psum_m = psA.tile([P, HC], F32, space="PSUM", tag="m")
            nc.tensor.matmul(psum_m[:], lhsT=identb[:], rhs=XL[:], start=True,
                             stop=False)
            nc.tensor.matmul(psum_m[:], lhsT=OHT[:], rhs=xr_blk[:],
                             start=False, stop=True)
            LR = sb.tile([P, HC], BF16, tag="LR")
            nc.scalar.activation(LR[:], psum_m[:], AF.Prelu, alpha=SLOPE)

            # attention scores: S[e, h] = sum_c LR[e, h, c] * att[h, c]
            TM = sb.tile([P, HC], BF16, tag="TM")
            nc.vector.tensor_mul(TM[:], LR[:], att_bc[:])
            S = sb.tile([P, H], F32, tag="S")
            nc.vector.tensor_reduce(
                out=S[:], in_=TM[:].rearrange("p (h c) -> p h c", h=H),
                axis=mybir.AxisListType.X, op=OP.add)
            P4f = sb.tile([P, H], F32, tag="P4f")
            nc.scalar.activation(P4f[:], S[:], AF.Exp)
            P4b = sb.tile([P, H], BF16, tag="P4b")
            nc.vector.tensor_copy(P4b[:], P4f[:])
            XLP = sb.tile([P, HC], BF16, tag="XLP")
            for h in range(H):
                nc.vector.tensor_scalar_mul(XLP[:, h * C:(h + 1) * C],
                                            XL[:, h * C:(h + 1) * C],
                                            P4f[:, h:h + 1])
            nc.tensor.matmul(psum_C[:], lhsT=OH[:], rhs=XLP[:], start=(t == 0),
                             stop=(t == t_max - 1))
            nc.tensor.matmul(psum_D[:], lhsT=OH[:], rhs=P4b[:], start=(t == 0),
                             stop=(t == t_max - 1))

        # block flush: OUT = C / (D + eps); BN moment accumulation
        Deps = sb.tile([P, H], F32, tag="Deps")
        nc.vector.tensor_scalar_add(Deps[:], psum_D[:], 1e-16)
        rec = sb.tile([P, H], F32, tag="rec")
        nc.vector.reciprocal(rec[:], Deps[:])
        OUT = sb.tile([P, HC], F32, tag="OUT")
        for h in range(H):
            nc.vector.tensor_scalar_mul(OUT[:, h * C:(h + 1) * C],
                                        psum_C[:, h * C:(h + 1) * C],
                                        rec[:, h:h + 1])
        nc.sync.dma_start(raw_dram[b * P:(b + 1) * P, :], OUT[:])
        SQ = sb.tile([P, HC], F32, tag="SQ")
        nc.scalar.activation(SQ[:], OUT[:], AF.Square)
        nc.tensor.matmul(psBN_sum[:], lhsT=ones_col[:], rhs=OUT[:],
                         start=(b == 0), stop=(b == NBLK_CORE - 1))
        nc.tensor.matmul(psBN_sq[:], lhsT=ones_col[:], rhs=SQ[:],
                         start=(b == 0), stop=(b == NBLK_CORE - 1))


def _bn_scale_shift(nc, hold, sb, psum_pool, stats_in_d, stats_out_d, psBN_sum,
                    psBN_sq, bng_row_d, bnb_row_d, bias_row_d, ones_row, tag,
                    collective_fn):
    """AllReduce BN moments across cores, compute broadcast scale/shift tiles.
    Small temps go in `sb` (transient pool); the returned broadcast tiles
    (scale_bc, shift_bc) [128, 512] live in `hold`."""
    stats = sb.tile([1, 2 * HC], F32, tag=f"st{tag}", bufs=1)
    nc.scalar.copy(stats[:, :HC], psBN_sum[:])
    nc.scalar.copy(stats[:, HC:], psBN_sq[:])
    nc.sync.dma_start(stats_in_d[:], stats[:])
    collective_fn("AllReduce", OP.add, [list(range(N_CORES))],
                  [stats_in_d[:]], [stats_out_d[:]])
    st = sb.tile([1, 2 * HC], F32, tag=f"str{tag}", bufs=1)
    nc.sync.dma_start(st[:], stats_out_d[:])

    bng = sb.tile([1, HC], F32, tag=f"bng{tag}", bufs=1)
    nc.sync.dma_start(bng[:], bng_row_d[:])
    bnb = sb.tile([1, HC], F32, tag=f"bnb{tag}", bufs=1)
    nc.sync.dma_start(bnb[:], bnb_row_d[:])
    bias = sb.tile([1, HC], F32, tag=f"bias{tag}", bufs=1)
    nc.sync.dma_start(bias[:], bias_row_d[:])

    inv_n = 1.0 / N
    mu0 = sb.tile([1, HC], F32, tag=f"mu0{tag}", bufs=1)
    nc.vector.tensor_scalar_mul(mu0[:], st[:, :HC], inv_n)
    ex2 = sb.tile([1, HC], F32, tag=f"ex2{tag}", bufs=1)
    nc.vector.tensor_scalar_mul(ex2[:], st[:, HC:], inv_n)
    mu0sq = sb.tile([1, HC], F32, tag=f"mu0sq{tag}", bufs=1)
    nc.vector.tensor_mul(mu0sq[:], mu0[:], mu0[:])
    var = sb.tile([1, HC], F32, tag=f"var{tag}", bufs=1)
    nc.vector.tensor_sub(var[:], ex2[:], mu0sq[:])
    vareps = sb.tile([1, HC], F32, tag=f"vareps{tag}", bufs=1)
    nc.vector.tensor_scalar_add(vareps[:], var[:], EPS_BN)
    sd = sb.tile([1, HC], F32, tag=f"sd{tag}", bufs=1)
    nc.scalar.activation(sd[:], vareps[:], AF.Sqrt)
    rsd = sb.tile([1, HC], F32, tag=f"rsd{tag}", bufs=1)
    nc.vector.reciprocal(rsd[:], sd[:])
    scale = sb.tile([1, HC], F32, tag=f"scale{tag}", bufs=1)
    nc.vector.tensor_mul(scale[:], bng[:], rsd[:])
    mup = sb.tile([1, HC], F32, tag=f"mup{tag}", bufs=1)
    nc.vector.tensor_add(mup[:], mu0[:], bias[:])
    t1 = sb.tile([1, HC], F32, tag=f"t1{tag}", bufs=1)
    nc.vector.tensor_mul(t1[:], mup[:], scale[:])
    shift = sb.tile([1, HC], F32, tag=f"shift{tag}", bufs=1)
    nc.vector.tensor_sub(shift[:], bnb[:], t1[:])

    ps_s = psum_pool.tile([P, HC], F32, space="PSUM", tag="m")
    nc.tensor.matmul(ps_s[:], lhsT=ones_row[:], rhs=scale[:], start=True,
                     stop=True)
    scale_bc = hold.tile([P, HC], F32, tag=f"scbc{tag}")
    nc.scalar.copy(scale_bc[:], ps_s[:])
    ps_h = psum_pool.tile([P, HC], F32, space="PSUM", tag="m")
    nc.tensor.matmul(ps_h[:], lhsT=ones_row[:], rhs=shift[:], start=True,
                     stop=True)
    shift_bc = hold.tile([P, HC], F32, tag=f"shbc{tag}")
    nc.scalar.copy(shift_bc[:], ps_h[:])
    return scale_bc, shift_bc


def _build_program(t_max):
    nc = bacc.Bacc("TRN2", target_bir_lowering=False, debug=False,
                   num_devices=N_CORES)

    def _collective(kind, op, groups_, ins, outs):
        nc.gpsimd.collective_compute(kind, op, replica_groups=groups_,
                                     ins=ins, outs=outs)

    # ---- I/O declarations -------------------------------------------------
    xTq_d = nc.dram_tensor("xTq_aug", [F_IN + 1, NSHARD], BF16, kind="ExternalInput")
    W1l_d = nc.dram_tensor("W1l_aug", [F_IN + 1, HC], BF16, kind="ExternalInput")
    W1r_d = nc.dram_tensor("W1r_aug", [F_IN + 1, HC], BF16, kind="ExternalInput")
    W2ls_d = nc.dram_tensor("W2l_shard", [W2_SH, HC], BF16, kind="ExternalInput")
    W2rs_d = nc.dram_tensor("W2r_shard", [W2_SH, HC], BF16, kind="ExternalInput")
    b2l_d = nc.dram_tensor("b2l_row", [1, HC], BF16, kind="ExternalInput")
    b2r_d = nc.dram_tensor("b2r_row", [1, HC], BF16, kind="ExternalInput")
    att1_d = nc.dram_tensor("att1_row", [1, HC], BF16, kind="ExternalInput")
    att2_d = nc.dram_tensor("att2_row", [1, HC], BF16, kind="ExternalInput")
    bn1g_d = nc.dram_tensor("bn1_g_row", [1, HC], F32, kind="ExternalInput")
    bn1b_d = nc.dram_tensor("bn1_b_row", [1, HC], F32, kind="ExternalInput")
    bias1_d = nc.dram_tensor("bias1_row", [1, HC], F32, kind="ExternalInput")
    bn2g_d = nc.dram_tensor("bn2_g_row", [1, HC], F32, kind="ExternalInput")
    bn2b_d = nc.dram_tensor("bn2_b_row", [1, HC], F32, kind="ExternalInput")
    bias2_d = nc.dram_tensor("bias2_row", [1, HC], F32, kind="ExternalInput")
    fc1s_d = nc.dram_tensor("fc1_shard", [FC1_SH, C], F32, kind="ExternalInput")
    fc2_d = nc.dram_tensor("fc2_w", [C, 1], F32, kind="ExternalInput")
    fc2b_d = nc.dram_tensor("fc2_b_col", [B, 1], F32, kind="ExternalInput")
    gfT_d = nc.dram_tensor("gfT", [G_DIM, B], F32, kind="ExternalInput")
    cntinv_d = nc.dram_tensor("cntinv_row", [1, B], F32, kind="ExternalInput")
    src_idx_d = nc.dram_tensor("src16", [NBLK_CORE, P, t_max], I16, kind="ExternalInput")
    dstl_d = nc.dram_tensor("dstl", [NBLK_CORE, P, t_max], U8, kind="ExternalInput")
    batch_d = nc.dram_tensor("batch_col", [NBLK_CORE, P, 1], U8, kind="ExternalInput")

    out_d = nc.dram_tensor("out_final", [B, 1], F32, kind="ExternalOutput")

    # internal DRAM
    am1s_d = nc.dram_tensor("am1s", [NSHARD, HC], BF16)
    am1_d = nc.dram_tensor("am1", [N_PAD, HC], BF16, addr_space="Shared")
    xr1_d = nc.dram_tensor("xr1", [NSHARD, HC], BF16)
    xr2_d = nc.dram_tensor("xr2", [NSHARD, HC], BF16)
    hT_d = nc.dram_tensor("hT", [HC, NSHARD], BF16)
    h1raw_d = nc.dram_tensor("h1raw", [NSHARD, HC], F32)
    am2s_d = nc.dram_tensor("am2s", [NSHARD, HC], BF16)
    am2_d = nc.dram_tensor("am2", [N_PAD, HC], BF16, addr_space="Shared")
    h2raw_d = nc.dram_tensor("h2raw", [NSHARD, HC], F32)
    W2l_full_d = nc.dram_tensor("W2l_full", [HC, HC], BF16, addr_space="Shared")
    W2r_full_d = nc.dram_tensor("W2r_full", [HC, HC], BF16, addr_space="Shared")
    fc1_full_d = nc.dram_tensor("fc1_full", [FC1_PAD, C], F32, addr_space="Shared")
    bn1in_d = nc.dram_tensor("bn1in", [1, 2 * HC], F32)
    bn1out_d = nc.dram_tensor("bn1out", [1, 2 * HC], F32, addr_space="Shared")
    bn2in_d = nc.dram_tensor("bn2in", [1, 2 * HC], F32)
    bn2out_d = nc.dram_tensor("bn2out", [1, 2 * HC], F32, addr_space="Shared")
    poolin_d = nc.dram_tensor("poolin", [H, P, B], F32)
    poolout_d = nc.dram_tensor("poolout", [H, P, B], F32, addr_space="Shared")
    W2l_in_d = nc.dram_tensor("W2l_in", [W2_SH, HC], BF16)
    W2r_in_d = nc.dram_tensor("W2r_in", [W2_SH, HC], BF16)
    fc1_in_d = nc.dram_tensor("fc1_in", [FC1_SH, C], F32)

    groups = [list(range(N_CORES))]

    with tile.TileContext(nc) as tc:
        # sharded-weight AllGathers: no deps, overlap with early compute
        # (collectives may not read IO tensors -> stage via internal DRAM)
        nc.sync.dma_start(W2l_in_d[:], W2ls_d[:])
        nc.sync.dma_start(W2r_in_d[:], W2rs_d[:])
        nc.sync.dma_start(fc1_in_d[:], fc1s_d[:])
        _collective("AllGather", OP.bypass, groups, [W2l_in_d[:]], [W2l_full_d[:]])
        _collective("AllGather", OP.bypass, groups, [W2r_in_d[:]], [W2r_full_d[:]])
        _collective("AllGather", OP.bypass, groups, [fc1_in_d[:]], [fc1_full_d[:]])

        with (
            tc.tile_pool(name="const", bufs=1) as cpool,
            tc.tile_pool(name="hold", bufs=1) as hold,
        ):
            # ---- P0: constants built on device --------------------------
            rowvals_b = cpool.tile([P, P], BF16)      # [p, f] = f
            nc.gpsimd.iota(rowvals_b[:], [[1, P]], channel_multiplier=0,
                           allow_small_or_imprecise_dtypes=True)
            rowvals_f = cpool.tile([P, P], F32)
            nc.gpsimd.iota(rowvals_f[:], [[1, P]], channel_multiplier=0,
                           allow_small_or_imprecise_dtypes=True)
            iota_col_f = cpool.tile([P, 1], F32)      # [p, 0] = p
            nc.gpsimd.iota(iota_col_f[:], [[0, 1]], channel_multiplier=1,
                           allow_small_or_imprecise_dtypes=True)
            ident = cpool.tile([P, P], F32)
            nc.vector.tensor_scalar(ident[:], rowvals_f[:], iota_col_f[:],
                                    None, OP.is_equal)
            identb = cpool.tile([P, P], BF16)
            nc.vector.tensor_scalar(identb[:], rowvals_b[:], iota_col_f[:],
                                    None, OP.is_equal)
            ones_row = cpool.tile([1, P], F32)
            nc.vector.memset(ones_row[:], 1.0)
            ones_rowb = cpool.tile([1, P], BF16)
            nc.vector.memset(ones_rowb[:], 1.0)
            ones_col = cpool.tile([P, 1], F32)
            nc.vector.memset(ones_col[:], 1.0)

            edge_consts = (identb, rowvals_b, ones_col)

            # att rows -> [P, HC] broadcast tiles (outer product with ones)
            with tc.tile_pool(name="p0ps", bufs=2, space="PSUM") as p0ps:
                att1_row = cpool.tile([1, HC], BF16)
                nc.sync.dma_start(att1_row[:], att1_d[:])
                att2_row = cpool.tile([1, HC], BF16)
                nc.sync.dma_start(att2_row[:], att2_d[:])
                ps_a1 = p0ps.tile([P, HC], F32, space="PSUM", tag="a")
                nc.tensor.matmul(ps_a1[:], lhsT=ones_rowb[:], rhs=att1_row[:],
                                 start=True, stop=True)
                att1_bc = cpool.tile([P, HC], BF16)
                nc.scalar.copy(att1_bc[:], ps_a1[:])
                ps_a2 = p0ps.tile([P, HC], F32, space="PSUM", tag="a")
                nc.tensor.matmul(ps_a2[:], lhsT=ones_rowb[:], rhs=att2_row[:],
                                 start=True, stop=True)
                att2_bc = cpool.tile([P, HC], BF16)
                nc.scalar.copy(att2_bc[:], ps_a2[:])

            # ---- P1: layer-1 node transforms (own shard only) -----------
            with (
                tc.tile_pool(name="p1sb", bufs=3) as p1sb,
                tc.tile_pool(name="p1ps", bufs=4, space="PSUM") as p1ps,
            ):
                W1l = p1sb.tile([F_IN + 1, HC], BF16, bufs=1)
                nc.sync.dma_start(W1l[:], W1l_d[:])
                W1r = p1sb.tile([F_IN + 1, HC], BF16, bufs=1)
                nc.sync.dma_start(W1r[:], W1r_d[:])
                for j in range(NBLK_CORE):
                    xtq = p1sb.tile([F_IN + 1, P], BF16, tag="xtq")
                    nc.sync.dma_start(xtq[:], xTq_d[:, j * P:(j + 1) * P])
                    ps = p1ps.tile([P, HC], F32, space="PSUM", tag="p1")
                    nc.tensor.matmul(ps[:], lhsT=xtq[:], rhs=W1l[:],
                                     start=True, stop=True)
                    ev = p1sb.tile([P, HC], BF16, tag="ev")
                    nc.scalar.copy(ev[:], ps[:])
                    nc.sync.dma_start(am1s_d[j * P:(j + 1) * P, :], ev[:])
                    ps2 = p1ps.tile([P, HC], F32, space="PSUM", tag="p1")
                    nc.tensor.matmul(ps2[:], lhsT=xtq[:], rhs=W1r[:],
                                     start=True, stop=True)
                    ev2 = p1sb.tile([P, HC], BF16, tag="ev2")
                    nc.scalar.copy(ev2[:], ps2[:])
                    nc.sync.dma_start(xr1_d[j * P:(j + 1) * P, :], ev2[:])

            # gather the full layer-1 source-transform table
            _collective("AllGather", OP.bypass, groups, [am1s_d[:]], [am1_d[:]])

            # ---- P2: layer-1 edge aggregation ---------------------------
            with (
                tc.tile_pool(name="e1sb", bufs=6) as esb,
                tc.tile_pool(name="e1psA", bufs=3, space="PSUM") as psA,
                tc.tile_pool(name="e1psT", bufs=1, space="PSUM") as psT,
                tc.tile_pool(name="e1psC", bufs=1, space="PSUM") as psC,
                tc.tile_pool(name="e1psD", bufs=1, space="PSUM") as psD,
                tc.tile_pool(name="e1psBN", bufs=1, space="PSUM") as psBN,
            ):
                psBN_sum = psBN.tile([1, HC], F32, space="PSUM")
                psBN_sq = psBN.tile([1, HC], F32, space="PSUM")
                _edge_layer(nc, (esb, psA, psT, psC, psD), t_max, am1_d,
                            xr1_d, att1_bc, h1raw_d, src_idx_d, dstl_d,
                            edge_consts, psBN_sum, psBN_sq)

                # ---- P3: BN1 stats + scale/shift ------------------------
                scale1_bc, shift1_bc = _bn_scale_shift(
                    nc, hold, esb, psA, bn1in_d, bn1out_d, psBN_sum, psBN_sq,
                    bn1g_d, bn1b_d, bias1_d, ones_row, "b1", _collective)

            # ---- P4: BN1 apply + relu + build hT ------------------------
            with (
                tc.tile_pool(name="p4sb", bufs=3) as p4sb,
                tc.tile_pool(name="p4ps", bufs=2, space="PSUM") as p4ps,
            ):
                for j in range(NBLK_CORE):
                    raw = p4sb.tile([P, HC], F32, tag="raw")
                    nc.sync.dma_start(raw[:], h1raw_d[j * P:(j + 1) * P, :])
                    t1 = p4sb.tile([P, HC], F32, tag="t1")
                    nc.vector.tensor_mul(t1[:], raw[:], scale1_bc[:])
                    t2 = p4sb.tile([P, HC], F32, tag="t2")
                    nc.vector.tensor_add(t2[:], t1[:], shift1_bc[:])
                    hsb = p4sb.tile([P, HC], F32, tag="h")
                    nc.vector.tensor_scalar_max(hsb[:], t2[:], 0.0)
                    hb = p4sb.tile([P, HC], BF16, tag="hb")
                    nc.scalar.copy(hb[:], hsb[:])
                    pst = p4ps.tile([P, HC], BF16, space="PSUM", tag="tr")
                    for ch in range(4):
                        nc.tensor.transpose(pst[:, ch * P:(ch + 1) * P],
                                            hb[:, ch * P:(ch + 1) * P],
                                            identb[:])
                    ev4 = p4sb.tile([P, HC], BF16, tag="ev4")
                    nc.scalar.copy(ev4[:], pst[:])
                    for ch in range(4):
                        nc.sync.dma_start(
                            hT_d[ch * P:(ch + 1) * P, j * P:(j + 1) * P],
                            ev4[:, ch * P:(ch + 1) * P])

            # ---- P5: layer-2 node transforms ----------------------------
            with (
                tc.tile_pool(name="p5sb", bufs=3) as p5sb,
                tc.tile_pool(name="p5w", bufs=1) as p5w,
                tc.tile_pool(name="p5ps", bufs=4, space="PSUM") as p5ps,
            ):
                W2l_sb = [p5w.tile([P, HC], BF16, name=f"W2l{k}", tag=f"W2l{k}")
                          for k in range(4)]
                W2r_sb = [p5w.tile([P, HC], BF16, name=f"W2r{k}", tag=f"W2r{k}")
                          for k in range(4)]
                for k in range(4):
                    nc.sync.dma_start(W2l_sb[k][:],
                                      W2l_full_d[k * P:(k + 1) * P, :])
                    nc.sync.dma_start(W2r_sb[k][:],
                                      W2r_full_d[k * P:(k + 1) * P, :])
                b2l = p5w.tile([1, HC], BF16)
                nc.sync.dma_start(b2l[:], b2l_d[:])
                b2r = p5w.tile([1, HC], BF16)
                nc.sync.dma_start(b2r[:], b2r_d[:])
                for j in range(NBLK_CORE):
                    hTj = []
                    for k in range(4):
                        hx = p5sb.tile([P, P], BF16, tag=f"hTj{k}",
                                       name=f"hTj{k}")
                        nc.sync.dma_start(
                            hx[:], hT_d[k * P:(k + 1) * P, j * P:(j + 1) * P])
                        hTj.append(hx)
                    psl = p5ps.tile([P, HC], F32, space="PSUM", tag="l")
                    for k in range(4):
                        nc.tensor.matmul(psl[:], lhsT=hTj[k][:],
                                         rhs=W2l_sb[k][:], start=(k == 0),
                                         stop=False)
                    nc.tensor.matmul(psl[:], lhsT=ones_rowb[:], rhs=b2l[:],
                                     start=False, stop=True)
                    ev = p5sb.tile([P, HC], BF16, tag="ev")
                    nc.scalar.copy(ev[:], psl[:])
                    nc.sync.dma_start(am2s_d[j * P:(j + 1) * P, :], ev[:])
                    psr = p5ps.tile([P, HC], F32, space="PSUM", tag="r")
                    for k in range(4):
                        nc.tensor.matmul(psr[:], lhsT=hTj[k][:],
                                         rhs=W2r_sb[k][:], start=(k == 0),
                                         stop=False)
                    nc.tensor.matmul(psr[:], lhsT=ones_rowb[:], rhs=b2r[:],
                                     start=False, stop=True)
                    ev5 = p5sb.tile([P, HC], BF16, tag="ev5")
                    nc.scalar.copy(ev5[:], psr[:])
                    nc.sync.dma_start(xr2_d[j * P:(j + 1) * P, :], ev5[:])

            # ---- P6: AllGather layer-2 source transforms ----------------
            _collective("AllGather", OP.bypass, groups, [am2s_d[:]], [am2_d[:]])

            # ---- P7: layer-2 edge aggregation ---------------------------
            with (
                tc.tile_pool(name="e2sb", bufs=6) as esb,
                tc.tile_pool(name="e2psA", bufs=3, space="PSUM") as psA,
                tc.tile_pool(name="e2psT", bufs=1, space="PSUM") as psT,
                tc.tile_pool(name="e2psC", bufs=1, space="PSUM") as psC,
                tc.tile_pool(name="e2psD", bufs=1, space="PSUM") as psD,
                tc.tile_pool(name="e2psBN", bufs=1, space="PSUM") as psBN,
            ):
                psBN_sum = psBN.tile([1, HC], F32, space="PSUM")
                psBN_sq = psBN.tile([1, HC], F32, space="PSUM")
                _edge_layer(nc, (esb, psA, psT, psC, psD), t_max, am2_d,
                            xr2_d, att2_bc, h2raw_d, src_idx_d, dstl_d,
                            edge_consts, psBN_sum, psBN_sq)
                scale2_bc, shift2_bc = _bn_scale_shift(
                    nc, hold, esb, psA, bn2in_d, bn2out_d, psBN_sum, psBN_sq,
                    bn2g_d, bn2b_d, bias2_d, ones_row, "b2", _collective)

            # ---- P8: BN2 apply + relu + pooling -------------------------
            with (
                tc.tile_pool(name="p8sb", bufs=3) as p8sb,
                tc.tile_pool(name="p8ps", bufs=1, space="PSUM") as p8ps,
            ):
                pool_ps = [p8ps.tile([P, B], F32, space="PSUM",
                                     name=f"pool{k}", tag=f"pool{k}")
                           for k in range(4)]
                for j in range(NBLK_CORE):
                    raw = p8sb.tile([P, HC], F32, tag="raw")
                    nc.sync.dma_start(raw[:], h2raw_d[j * P:(j + 1) * P, :])
                    t1 = p8sb.tile([P, HC], F32, tag="t1")
                    nc.vector.tensor_mul(t1[:], raw[:], scale2_bc[:])
                    t2 = p8sb.tile([P, HC], F32, tag="t2")
                    nc.vector.tensor_add(t2[:], t1[:], shift2_bc[:])
                    hsb = p8sb.tile([P, HC], F32, tag="h")
                    nc.vector.tensor_scalar_max(hsb[:], t2[:], 0.0)
                    bat8 = p8sb.tile([P, 1], U8, tag="bat8")
                    nc.sync.dma_start(bat8[:], batch_d[j])
                    batch_sb = p8sb.tile([P, 1], F32, tag="bat")
                    nc.vector.tensor_copy(batch_sb[:], bat8[:])
                    ohb = p8sb.tile([P, B], F32, tag="ohb")
                    nc.vector.tensor_scalar(ohb[:], rowvals_f[:, :B],
                                            batch_sb[:], None, OP.is_equal)
                    for ch in range(4):
                        nc.tensor.matmul(pool_ps[ch][:],
                                         lhsT=hsb[:, ch * P:(ch + 1) * P],
                                         rhs=ohb[:], start=(j == 0),
                                         stop=(j == NBLK_CORE - 1))
                poolsb = p8sb.tile([P, 4 * B], F32)
                for ch in range(4):
                    nc.scalar.copy(poolsb[:, ch * B:(ch + 1) * B],
                                   pool_ps[ch][:])
                for ch in range(4):
                    nc.sync.dma_start(poolin_d[ch],
                                      poolsb[:, ch * B:(ch + 1) * B])
                _collective("AllReduce", OP.add, groups,
                            [poolin_d[:]], [poolout_d[:]])

            # ---- P9: head -----------------------------------------------
            with (
                tc.tile_pool(name="p9sb", bufs=1) as p9sb,
                tc.tile_pool(name="p9ps", bufs=1, space="PSUM") as p9ps,
            ):
                ci = p9sb.tile([1, B], F32)
                nc.sync.dma_start(ci[:], cntinv_d[:])
                ps_ci = p9ps.tile([P, B], F32, space="PSUM", tag="ci")
                nc.tensor.matmul(ps_ci[:], lhsT=ones_row[:], rhs=ci[:],
                                 start=True, stop=True)
                cib = p9sb.tile([P, B], F32)
                nc.scalar.copy(cib[:], ps_ci[:])

                zc = []
                for ch in range(4):
                    pc = p9sb.tile([P, B], F32, tag=f"pc{ch}")
                    nc.sync.dma_start(pc[:], poolout_d[ch])
                    z = p9sb.tile([P, B], F32, tag=f"z{ch}")
                    nc.vector.tensor_mul(z[:], pc[:], cib[:])
                    zc.append(z)
                gfT = p9sb.tile([G_DIM, B], F32)
                nc.sync.dma_start(gfT[:], gfT_d[:])
                fc1 = []
                for ch in range(4):
                    w = p9sb.tile([P, C], F32, tag=f"w{ch}")
                    nc.sync.dma_start(w[:], fc1_full_d[ch * P:(ch + 1) * P, :])
                    fc1.append(w)
                fc1g = p9sb.tile([G_DIM, C], F32)
                nc.sync.dma_start(fc1g[:], fc1_full_d[HC:HC + G_DIM, :])
                fc1b = p9sb.tile([1, C], F32)
                nc.sync.dma_start(fc1b[:],
                                  fc1_full_d[HC + G_DIM:HC + G_DIM + 1, :])

                ps_z1 = p9ps.tile([B, C], F32, space="PSUM", tag="z1")
                for ch in range(4):
                    nc.tensor.matmul(ps_z1[:], lhsT=zc[ch][:], rhs=fc1[ch][:],
                                     start=(ch == 0), stop=False)
                nc.tensor.matmul(ps_z1[:], lhsT=gfT[:], rhs=fc1g[:],
                                 start=False, stop=False)
                nc.tensor.matmul(ps_z1[:], lhsT=ones_row[:, :B], rhs=fc1b[:],
                                 start=False, stop=True)
                z1 = p9sb.tile([B, C], F32)
                nc.scalar.activation(z1[:], ps_z1[:], AF.Relu)

                ps_z1T = p9ps.tile([C, B], F32, space="PSUM", tag="z1T")
                nc.tensor.transpose(ps_z1T[:], z1[:], ident[:B, :B])
                z1T = p9sb.tile([C, B], F32)
                nc.scalar.copy(z1T[:], ps_z1T[:])

                fc2 = p9sb.tile([C, 1], F32)
                nc.sync.dma_start(fc2[:], fc2_d[:])
                ps_o = p9ps.tile([B, 1], F32, space="PSUM", tag="o")
                nc.tensor.matmul(ps_o[:], lhsT=z1T[:], rhs=fc2[:], start=True,
                                 stop=True)
                fc2b = p9sb.tile([B, 1], F32)
                nc.sync.dma_start(fc2b[:], fc2b_d[:])
                osb = p9sb.tile([B, 1], F32)
                nc.vector.tensor_scalar_add(osb[:], ps_o[:], fc2b[:])
                nc.sync.dma_start(out_d[:], osb[:])

    nc.compile()
    return nc


def _make_runner(nc):
    """Jit the PJRT executable once; returns run(in_maps) -> out_final[B]."""
    install_neuronx_cc_hook()
    partition_name = (nc.partition_id_tensor.name
                      if nc.partition_id_tensor else None)
    in_names, out_names, out_avals, zero_outs = [], [], [], []
    for alloc in nc.m.functions[0].allocations:
        if not isinstance(alloc, mybir.MemoryLocationSet):
            continue
        name = alloc.memorylocations[0].name
        if alloc.kind == "ExternalInput":
            if name != partition_name:
                in_names.append(name)
        elif alloc.kind == "ExternalOutput":
            out_names.append(name)
            shape = tuple(alloc.tensor_shape)
            dtype = mybir.dt.np(alloc.dtype)
            out_avals.append(jax.core.ShapedArray(shape, dtype))
            zero_outs.append(np.zeros(shape, dtype))
    n_params = len(in_names)
    n_outs = len(out_avals)
    in_names.extend(out_names)
    if partition_name is not None:
        in_names.append(partition_name)
    donate = tuple(range(n_params, n_params + n_outs))

    def _body(*args):
        operands = list(args)
        if partition_name is not None:
            operands.append(partition_id_tensor())
        return tuple(_bass_exec_p.bind(
            *operands, out_avals=tuple(out_avals), in_names=tuple(in_names),
            out_names=tuple(out_names), lowering_input_output_aliases=(),
            sim_require_finite=True, sim_require_nnan=True, nc=nc))

    devices = jax.devices()[:N_CORES]
    mesh = Mesh(np.asarray(devices), ("core",))
    in_specs = (PartitionSpec("core"),) * (n_params + n_outs)
    out_specs = (PartitionSpec("core"),) * len(out_names)
    sharded = jax.jit(
        shard_map(_body, mesh=mesh, in_specs=in_specs, out_specs=out_specs,
                  check_rep=False),
        donate_argnums=donate, keep_unused=True)
    param_names = in_names[:n_params]

    def run(in_maps):
        concat_in = [
            np.concatenate([np.asarray(m[name]) for m in in_maps], axis=0)
            for name in param_names
        ]
        concat_zeros = [
            np.zeros((N_CORES * z.shape[0], *z.shape[1:]), z.dtype)
            for z in zero_outs
        ]
        out_arrs = sharded(*concat_in, *concat_zeros)
        oi = out_names.index("out_final")
        return np.asarray(out_arrs[oi])[:B].reshape(B)

    return run


def _preprocess(inputs):
    """Host-side: edge sorting/sharding/padding + weight repacking."""
    x = np.asarray(inputs["x"], np.float32)
    gf = np.asarray(inputs["global_feat"], np.float32)
    ei = np.asarray(inputs["edge_index"])
    batch = np.asarray(inputs["batch"]).astype(np.int64)

    loops = np.arange(N, dtype=np.int32)
    src = np.concatenate([ei[0].astype(np.int32, copy=False), loops])
    dst = np.concatenate([ei[1].astype(np.int32, copy=False), loops])
    blk16 = (dst >> 7).astype(np.int16)
    order = np.argsort(blk16, kind="stable")  # radix sort on int16 keys
    src, dst = src[order], dst[order]
    blk = dst >> 7
    counts = np.bincount(blk, minlength=NBLK)
    t_max = max(1, int(np.ceil(counts.max() / P)))
    e_cap = t_max * P

    starts = np.concatenate([[0], np.cumsum(counts)])
    pos = np.arange(dst.shape[0]) - starts[blk]
    src_pad = np.zeros((NBLK, e_cap), np.int16)
    src_pad[blk, pos] = src.astype(np.int16)
    dstl_pad = np.full((NBLK, e_cap), 200, np.uint8)
    dstl_pad[blk, pos] = (dst & 127).astype(np.uint8)
    # [blk, e_cap] -> [blk, 128, t_max] with edge e of tile t at [e, t]
    src_t = np.ascontiguousarray(
        src_pad.reshape(NBLK, t_max, P).transpose(0, 2, 1))
    dstl_t = np.ascontiguousarray(
        dstl_pad.reshape(NBLK, t_max, P).transpose(0, 2, 1))

    xT_aug = np.zeros((F_IN + 1, N_PAD), np.float32)
    xT_aug[:F_IN, :N] = x.T
    xT_aug[F_IN, :] = 1.0
    xT_aug = xT_aug.astype("bfloat16")

    def aug_w(w, bvec):
        return np.concatenate([np.asarray(w, np.float32),
                               np.asarray(bvec, np.float32)[None, :]],
                              0).astype("bfloat16")

    W2l = np.asarray(inputs["W2l"], np.float32).astype("bfloat16")
    W2r = np.asarray(inputs["W2r"], np.float32).astype("bfloat16")

    fc1_pad = np.zeros((FC1_PAD, C), np.float32)
    fc1_pad[:HC] = np.asarray(inputs["fc1_w"], np.float32)[:HC]
    fc1_pad[HC:HC + G_DIM] = np.asarray(inputs["fc1_w"], np.float32)[HC:]
    fc1_pad[HC + G_DIM] = np.asarray(inputs["fc1_b"], np.float32)

    cnt = np.bincount(batch, minlength=B).astype(np.float32)
    cntinv = (1.0 / np.maximum(cnt, 1.0)).reshape(1, B)

    batch_p = np.full(N_PAD, 255, np.uint8)
    batch_p[:N] = batch.astype(np.uint8)
    batch_col = batch_p.reshape(NBLK, P, 1)

    common = {
        "W1l_aug": aug_w(inputs["W1l"], inputs["b1l"]),
        "W1r_aug": aug_w(inputs["W1r"], inputs["b1r"]),
        "b2l_row": np.asarray(inputs["b2l"], np.float32).reshape(1, HC).astype("bfloat16"),
        "b2r_row": np.asarray(inputs["b2r"], np.float32).reshape(1, HC).astype("bfloat16"),
        "att1_row": np.asarray(inputs["att1"], np.float32).reshape(1, HC).astype("bfloat16"),
        "att2_row": np.asarray(inputs["att2"], np.float32).reshape(1, HC).astype("bfloat16"),
        "bn1_g_row": np.asarray(inputs["bn1_g"], np.float32).reshape(1, HC),
        "bn1_b_row": np.asarray(inputs["bn1_b"], np.float32).reshape(1, HC),
        "bias1_row": np.asarray(inputs["bias1"], np.float32).reshape(1, HC),
        "bn2_g_row": np.asarray(inputs["bn2_g"], np.float32).reshape(1, HC),
        "bn2_b_row": np.asarray(inputs["bn2_b"], np.float32).reshape(1, HC),
        "bias2_row": np.asarray(inputs["bias2"], np.float32).reshape(1, HC),
        "fc2_w": np.asarray(inputs["fc2_w"], np.float32).reshape(C, 1),
        "fc2_b_col": np.full((B, 1), np.asarray(inputs["fc2_b"], np.float32).reshape(-1)[0], np.float32),
        "gfT": np.ascontiguousarray(gf.T),
        "cntinv_row": cntinv,
    }

    in_maps = []
    for c in range(N_CORES):
        lo, hi = c * NBLK_CORE, (c + 1) * NBLK_CORE
        m = dict(common)
        m["xTq_aug"] = np.ascontiguousarray(xT_aug[:, lo * P:hi * P])
        m["W2l_shard"] = W2l[c * W2_SH:(c + 1) * W2_SH]
        m["W2r_shard"] = W2r[c * W2_SH:(c + 1) * W2_SH]
        m["fc1_shard"] = fc1_pad[c * FC1_SH:(c + 1) * FC1_SH]
        m["src16"] = src_t[lo:hi]
        m["dstl"] = dstl_t[lo:hi]
        m["batch_col"] = batch_col[lo:hi]
        in_maps.append(m)
    return in_maps, t_max


class _FastResult:
    exec_time_ns = None


def _run(inputs, trace=False):
    in_maps, t_max = _preprocess(inputs)
    if t_max not in _PROGRAM_CACHE:
        nc = _build_program(t_max)
        runner = _make_runner(nc)
        _PROGRAM_CACHE[t_max] = (nc, runner)
    nc, runner = _PROGRAM_CACHE[t_max]
    if trace:
        res = run_bass_kernel_spmd(nc, in_maps, list(range(N_CORES)),
                                   trace=True)
        out = np.asarray(res.results[0]["out_final"], np.float32).reshape(B)
        return out, res
    out = runner(in_maps)
    return np.asarray(out, np.float32), _FastResult()


def kernel(**inputs) -> np.ndarray:
    out, _ = _run(inputs, trace=False)
    return out


# revision 12
# speedup vs baseline: 32.7898x; 1.0471x over previous
"""GATv2WithGlobal Trainium2 kernel — 8-core SPMD bass implementation.

Strategy (dst-sharded message passing, transfer-minimized):
- Nodes padded 30000->30720, sharded as 8 cores x 30 blocks x 128 dst nodes.
- Edges (+self loops) grouped per dst-block (radix sort on int16 block keys),
  padded to a uniform t_max tiles of 128 edges per block (SPMD needs one
  program for all cores).
- Host ships ~0.4MB/core packed into 4 arrays (bf16/f32/u8 blobs + int16
  source indices); the kernel unpacks them with strided access patterns.
  All one-hot scatter/gather matrices, identity matrices and iota constants
  are built ON DEVICE (iota + is_equal). Layer-1 source transforms are
  computed per-shard then AllGathered; W2/fc1 weights are shipped sharded
  and AllGathered on device — nothing large crosses the host link.
- Per edge tile: indirect-DMA gather of source features; OH[e,d]=(dst_l[e]==d)
  via vector is_equal against an iota row-values constant; OHT via PE
  transpose; one-hot matmuls for target-feature broadcast and
  scatter-aggregation with PSUM accumulation; LeakyReLU on ScalarE; attention
  scores via broadcast-mul + strided reduce on VectorE; exp on ScalarE.
  Segment softmax without max-subtraction (scores are O(few), fp32-safe).
- BatchNorm via E[x^2]-mu^2 with the layer bias folded into the stats
  (variance is bias-invariant); BN applied with broadcast scale/shift.
- Cross-core exchanges: AllGather for sharded weights + source-transform
  tables, AllReduce for BN stats & pooled features.
- The PJRT executable is jitted once per program and cached; per-call work is
  host preprocessing (~15ms) + ~3MB H2D + execute.
"""

import numpy as np

import jax
from jax.sharding import Mesh, PartitionSpec
from jax.experimental.shard_map import shard_map

import concourse.bass as bass
import concourse.mybir as mybir
import concourse.tile as tile
from concourse import bacc
from concourse.bass_utils import run_bass_kernel_spmd
from concourse.bass2jax import (
    _bass_exec_p,
    install_neuronx_cc_hook,
    partition_id_tensor,
)

# problem dims (hardcoded per contract)
N = 30000
N_PAD = 30720
P = 128
N_CORES = 8
NBLK = N_PAD // P            # 240
NBLK_CORE = NBLK // N_CORES  # 30
NSHARD = NBLK_CORE * P       # 3840
H, C, HC = 4, 128, 512
F_IN, G_DIM, B = 9, 50, 64
W2_SH = HC // N_CORES        # 64 rows of W2 shipped per core
FC1_ROWS = HC + G_DIM + 1    # 563
FC1_PAD = 568                # 8 * 71
FC1_SH = FC1_PAD // N_CORES  # 71
SLOPE = 0.2
EPS_BN = 1e-5

F32 = mybir.dt.float32
BF16 = mybir.dt.bfloat16
I32 = mybir.dt.int32
I16 = mybir.dt.int16
U8 = mybir.dt.uint8
AF = mybir.ActivationFunctionType
OP = mybir.AluOpType

_PROGRAM_CACHE: dict = {}

# ---- packed-input layouts (element offsets, shared by host + device) -------
_B16_FIELDS = [
    ("xTq", (F_IN + 1) * NSHARD), ("W1l", (F_IN + 1) * HC),
    ("W1r", (F_IN + 1) * HC), ("W2l", W2_SH * HC), ("W2r", W2_SH * HC),
    ("b2l", HC), ("b2r", HC), ("att1", HC), ("att2", HC),
]
_F32_FIELDS = [
    ("bn1g", HC), ("bn1b", HC), ("bias1", HC),
    ("bn2g", HC), ("bn2b", HC), ("bias2", HC),
    ("cntinv", B), ("fc2w", C), ("fc2b", B),
    ("fc1", FC1_SH * C), ("gfT", G_DIM * B),
]


def _layout(fields):
    off, o = {}, 0
    for name, size in fields:
        off[name] = o
        o += size
    return off, o


B16_OFF, B16_SIZE = _layout(_B16_FIELDS)
F32_OFF, F32_SIZE = _layout(_F32_FIELDS)


def _edge_layer(nc, pools, t_max, am_dram, xr_dram, att_bc, raw_dram,
                src_idx_d, dstl_aps, consts, psBN_sum, psBN_sq):
    """One GATv2 message-passing layer over this core's 30 dst blocks (bf16).
    Writes aggregated (pre-BN, bias-free) features to raw_dram (f32) and
    accumulates BN sum/sumsq into the two persistent PSUM tiles."""
    identb, rowvals_b, ones_col = consts
    sb, psA, psT, psC, psD = pools

    for b in range(NBLK_CORE):
        s16 = sb.tile([P, t_max], I16, tag="s16", bufs=2)
        nc.sync.dma_start(s16[:], src_idx_d[b])
        idx_sb = sb.tile([P, t_max], I32, tag="idx", bufs=2)
        nc.vector.tensor_copy(idx_sb[:], s16[:])
        d8 = sb.tile([P, t_max], U8, tag="d8", bufs=2)
        nc.sync.dma_start(d8[:], dstl_aps[b])
        dstl_sb = sb.tile([P, t_max], F32, tag="dstl", bufs=2)
        nc.vector.tensor_copy(dstl_sb[:], d8[:])
        xr_blk = sb.tile([P, HC], BF16, tag="xrb", bufs=2)
        nc.sync.dma_start(xr_blk[:], xr_dram[b * P:(b + 1) * P, :])

        psum_C = psC.tile([P, HC], F32, space="PSUM", tag="C")
        psum_D = psD.tile([P, H], F32, space="PSUM", tag="D")

        for t in range(t_max):
            # one-hot OH[e, d] = (dst_local[e] == d), built on device
            OH = sb.tile([P, P], BF16, tag="OH")
            nc.vector.tensor_scalar(OH[:], rowvals_b[:], dstl_sb[:, t:t + 1],
                                    None, OP.is_equal)
            ps_T = psT.tile([P, P], BF16, space="PSUM", tag="T")
            nc.tensor.transpose(ps_T[:], OH[:], identb[:])
            OHT = sb.tile([P, P], BF16, tag="OHT")
            nc.scalar.copy(OHT[:], ps_T[:])

            XL = sb.tile([P, HC], BF16, tag="XL")
            nc.gpsimd.indirect_dma_start(
                out=XL[:], out_offset=None, in_=am_dram[:],
                in_offset=bass.IndirectOffsetOnAxis(ap=idx_sb[:, t:t + 1],
                                                    axis=0),
            )

            psum_m = psA.tile([P, HC], F32, space="PSUM", tag="m")
            nc.tensor.matmul(psum_m[:], lhsT=identb[:], rhs=XL[:], start=True,
                             stop=False)
            nc.tensor.matmul(psum_m[:], lhsT=OHT[:], rhs=xr_blk[:],
                             start=False, stop=True)
            LR = sb.tile([P, HC], BF16, tag="LR")
            nc.scalar.activation(LR[:], psum_m[:], AF.Prelu, alpha=SLOPE)

            # attention scores: S[e, h] = sum_c LR[e, h, c] * att[h, c]
            TM = sb.tile([P, HC], BF16, tag="TM")
            nc.vector.tensor_mul(TM[:], LR[:], att_bc[:])
            S = sb.tile([P, H], F32, tag="S")
            nc.vector.tensor_reduce(
                out=S[:], in_=TM[:].rearrange("p (h c) -> p h c", h=H),
                axis=mybir.AxisListType.X, op=OP.add)
            P4f = sb.tile([P, H], F32, tag="P4f")
            nc.scalar.activation(P4f[:], S[:], AF.Exp)
            P4b = sb.tile([P, H], BF16, tag="P4b")
            nc.vector.tensor_copy(P4b[:], P4f[:])
            XLP = sb.tile([P, HC], BF16, tag="XLP")
            for h in range(H):
                nc.vector.tensor_scalar_mul(XLP[:, h * C:(h + 1) * C],
                                            XL[:, h * C:(h + 1) * C],
                                            P4f[:, h:h + 1])
            nc.tensor.matmul(psum_C[:], lhsT=OH[:], rhs=XLP[:], start=(t == 0),
                             stop=(t == t_max - 1))
            nc.tensor.matmul(psum_D[:], lhsT=OH[:], rhs=P4b[:], start=(t == 0),
                             stop=(t == t_max - 1))

        # block flush: OUT = C / (D + eps); BN moment accumulation
        Deps = sb.tile([P, H], F32, tag="Deps")
        nc.vector.tensor_scalar_add(Deps[:], psum_D[:], 1e-16)
        rec = sb.tile([P, H], F32, tag="rec")
        nc.vector.reciprocal(rec[:], Deps[:])
        OUT = sb.tile([P, HC], F32, tag="OUT")
        for h in range(H):
            nc.vector.tensor_scalar_mul(OUT[:, h * C:(h + 1) * C],
                                        psum_C[:, h * C:(h + 1) * C],
                                        rec[:, h:h + 1])
        nc.sync.dma_start(raw_dram[b * P:(b + 1) * P, :], OUT[:])
        SQ = sb.tile([P, HC], F32, tag="SQ")
        nc.scalar.activation(SQ[:], OUT[:], AF.Square)
        nc.tensor.matmul(psBN_sum[:], lhsT=ones_col[:], rhs=OUT[:],
                         start=(b == 0), stop=(b == NBLK_CORE - 1))
        nc.tensor.matmul(psBN_sq[:], lhsT=ones_col[:], rhs=SQ[:],
                         start=(b == 0), stop=(b == NBLK_CORE - 1))


def _bn_scale_shift(nc, hold, sb, psum_pool, stats_in_d, stats_out_d, psBN_sum,
                    psBN_sq, bng_ap, bnb_ap, bias_ap, ones_row, tag,
                    collective_fn):
    """AllReduce BN moments across cores, compute broadcast scale/shift tiles.
    Small temps go in `sb` (transient pool); the returned broadcast tiles
    (scale_bc, shift_bc) [128, 512] live in `hold`."""
    stats = sb.tile([1, 2 * HC], F32, tag=f"st{tag}", bufs=1)
    nc.scalar.copy(stats[:, :HC], psBN_sum[:])
    nc.scalar.copy(stats[:, HC:], psBN_sq[:])
    nc.sync.dma_start(stats_in_d[:], stats[:])
    collective_fn("AllReduce", OP.add, [list(range(N_CORES))],
                  [stats_in_d[:]], [stats_out_d[:]])
    st = sb.tile([1, 2 * HC], F32, tag=f"str{tag}", bufs=1)
    nc.sync.dma_start(st[:], stats_out_d[:])

    bng = sb.tile([1, HC], F32, tag=f"bng{tag}", bufs=1)
    nc.sync.dma_start(bng[:], bng_ap)
    bnb = sb.tile([1, HC], F32, tag=f"bnb{tag}", bufs=1)
    nc.sync.dma_start(bnb[:], bnb_ap)
    bias = sb.tile([1, HC], F32, tag=f"bias{tag}", bufs=1)
    nc.sync.dma_start(bias[:], bias_ap)

    inv_n = 1.0 / N
    mu0 = sb.tile([1, HC], F32, tag=f"mu0{tag}", bufs=1)
    nc.vector.tensor_scalar_mul(mu0[:], st[:, :HC], inv_n)
    ex2 = sb.tile([1, HC], F32, tag=f"ex2{tag}", bufs=1)
    nc.vector.tensor_scalar_mul(ex2[:], st[:, HC:], inv_n)
    mu0sq = sb.tile([1, HC], F32, tag=f"mu0sq{tag}", bufs=1)
    nc.vector.tensor_mul(mu0sq[:], mu0[:], mu0[:])
    var = sb.tile([1, HC], F32, tag=f"var{tag}", bufs=1)
    nc.vector.tensor_sub(var[:], ex2[:], mu0sq[:])
    vareps = sb.tile([1, HC], F32, tag=f"vareps{tag}", bufs=1)
    nc.vector.tensor_scalar_add(vareps[:], var[:], EPS_BN)
    sd = sb.tile([1, HC], F32, tag=f"sd{tag}", bufs=1)
    nc.scalar.activation(sd[:], vareps[:], AF.Sqrt)
    rsd = sb.tile([1, HC], F32, tag=f"rsd{tag}", bufs=1)
    nc.vector.reciprocal(rsd[:], sd[:])
    scale = sb.tile([1, HC], F32, tag=f"scale{tag}", bufs=1)
    nc.vector.tensor_mul(scale[:], bng[:], rsd[:])
    mup = sb.tile([1, HC], F32, tag=f"mup{tag}", bufs=1)
    nc.vector.tensor_add(mup[:], mu0[:], bias[:])
    t1 = sb.tile([1, HC], F32, tag=f"t1{tag}", bufs=1)
    nc.vector.tensor_mul(t1[:], mup[:], scale[:])
    shift = sb.tile([1, HC], F32, tag=f"shift{tag}", bufs=1)
    nc.vector.tensor_sub(shift[:], bnb[:], t1[:])

    ps_s = psum_pool.tile([P, HC], F32, space="PSUM", tag="m")
    nc.tensor.matmul(ps_s[:], lhsT=ones_row[:], rhs=scale[:], start=True,
                     stop=True)
    scale_bc = hold.tile([P, HC], F32, tag=f"scbc{tag}")
    nc.scalar.copy(scale_bc[:], ps_s[:])
    ps_h = psum_pool.tile([P, HC], F32, space="PSUM", tag="m")
    nc.tensor.matmul(ps_h[:], lhsT=ones_row[:], rhs=shift[:], start=True,
                     stop=True)
    shift_bc = hold.tile([P, HC], F32, tag=f"shbc{tag}")
    nc.scalar.copy(shift_bc[:], ps_h[:])
    return scale_bc, shift_bc


def _build_program(t_max):
    nc = bacc.Bacc("TRN2", target_bir_lowering=False, debug=False,
                   num_devices=N_CORES)

    def _collective(kind, op, groups_, ins, outs):
        nc.gpsimd.collective_compute(kind, op, replica_groups=groups_,
                                     ins=ins, outs=outs)

    # ---- I/O declarations: 4 packed inputs per core -----------------------
    b16_d = nc.dram_tensor("blob_b16", [B16_SIZE], BF16, kind="ExternalInput")
    f32_d = nc.dram_tensor("blob_f32", [F32_SIZE], F32, kind="ExternalInput")
    u8_size = NBLK_CORE * P * t_max + NBLK_CORE * P
    u8_d = nc.dram_tensor("blob_u8", [u8_size], U8, kind="ExternalInput")
    src_idx_d = nc.dram_tensor("src16", [NBLK_CORE, P, t_max], I16,
                               kind="ExternalInput")
    out_d = nc.dram_tensor("out_final", [B, 1], F32, kind="ExternalOutput")

    def b16(name, shape):
        o = B16_OFF[name]
        if len(shape) == 1:
            return bass.AP(b16_d, o, [[shape[0], 1], [1, shape[0]]])
        return bass.AP(b16_d, o, [[shape[1], shape[0]], [1, shape[1]]])

    def f32(name, shape):
        o = F32_OFF[name]
        if len(shape) == 1:
            return bass.AP(f32_d, o, [[shape[0], 1], [1, shape[0]]])
        return bass.AP(f32_d, o, [[shape[1], shape[0]], [1, shape[1]]])

    dstl_aps = [bass.AP(u8_d, b * P * t_max, [[t_max, P], [1, t_max]])
                for b in range(NBLK_CORE)]
    bat_off = NBLK_CORE * P * t_max
    batch_aps = [bass.AP(u8_d, bat_off + b * P, [[1, P], [1, 1]])
                 for b in range(NBLK_CORE)]

    # internal DRAM
    am1s_d = nc.dram_tensor("am1s", [NSHARD, HC], BF16)
    am1_d = nc.dram_tensor("am1", [N_PAD, HC], BF16, addr_space="Shared")
    xr1_d = nc.dram_tensor("xr1", [NSHARD, HC], BF16)
    xr2_d = nc.dram_tensor("xr2", [NSHARD, HC], BF16)
    hT_d = nc.dram_tensor("hT", [HC, NSHARD], BF16)
    h1raw_d = nc.dram_tensor("h1raw", [NSHARD, HC], F32)
    am2s_d = nc.dram_tensor("am2s", [NSHARD, HC], BF16)
    am2_d = nc.dram_tensor("am2", [N_PAD, HC], BF16, addr_space="Shared")
    h2raw_d = nc.dram_tensor("h2raw", [NSHARD, HC], F32)
    W2l_in_d = nc.dram_tensor("W2l_in", [W2_SH, HC], BF16)
    W2r_in_d = nc.dram_tensor("W2r_in", [W2_SH, HC], BF16)
    fc1_in_d = nc.dram_tensor("fc1_in", [FC1_SH, C], F32)
    W2l_full_d = nc.dram_tensor("W2l_full", [HC, HC], BF16, addr_space="Shared")
    W2r_full_d = nc.dram_tensor("W2r_full", [HC, HC], BF16, addr_space="Shared")
    fc1_full_d = nc.dram_tensor("fc1_full", [FC1_PAD, C], F32, addr_space="Shared")
    bn1in_d = nc.dram_tensor("bn1in", [1, 2 * HC], F32)
    bn1out_d = nc.dram_tensor("bn1out", [1, 2 * HC], F32, addr_space="Shared")
    bn2in_d = nc.dram_tensor("bn2in", [1, 2 * HC], F32)
    bn2out_d = nc.dram_tensor("bn2out", [1, 2 * HC], F32, addr_space="Shared")
    poolin_d = nc.dram_tensor("poolin", [H, P, B], F32)
    poolout_d = nc.dram_tensor("poolout", [H, P, B], F32, addr_space="Shared")

    groups = [list(range(N_CORES))]

    with tile.TileContext(nc) as tc:
        # sharded-weight AllGathers: no deps, overlap with early compute
        # (collectives may not read IO tensors -> stage via internal DRAM)
        nc.sync.dma_start(W2l_in_d[:], b16("W2l", (W2_SH, HC)))
        nc.sync.dma_start(W2r_in_d[:], b16("W2r", (W2_SH, HC)))
        nc.sync.dma_start(fc1_in_d[:], f32("fc1", (FC1_SH, C)))
        _collective("AllGather", OP.bypass, groups, [W2l_in_d[:]], [W2l_full_d[:]])
        _collective("AllGather", OP.bypass, groups, [W2r_in_d[:]], [W2r_full_d[:]])
        _collective("AllGather", OP.bypass, groups, [fc1_in_d[:]], [fc1_full_d[:]])

        with (
            tc.tile_pool(name="const", bufs=1) as cpool,
            tc.tile_pool(name="hold", bufs=1) as hold,
        ):
            # ---- P0: constants built on device --------------------------
            rowvals_b = cpool.tile([P, P], BF16)      # [p, f] = f
            nc.gpsimd.iota(rowvals_b[:], [[1, P]], channel_multiplier=0,
                           allow_small_or_imprecise_dtypes=True)
            rowvals_f = cpool.tile([P, P], F32)
            nc.gpsimd.iota(rowvals_f[:], [[1, P]], channel_multiplier=0,
                           allow_small_or_imprecise_dtypes=True)
            iota_col_f = cpool.tile([P, 1], F32)      # [p, 0] = p
            nc.gpsimd.iota(iota_col_f[:], [[0, 1]], channel_multiplier=1,
                           allow_small_or_imprecise_dtypes=True)
            ident = cpool.tile([P, P], F32)
            nc.vector.tensor_scalar(ident[:], rowvals_f[:], iota_col_f[:],
                                    None, OP.is_equal)
            identb = cpool.tile([P, P], BF16)
            nc.vector.tensor_scalar(identb[:], rowvals_b[:], iota_col_f[:],
                                    None, OP.is_equal)
            ones_row = cpool.tile([1, P], F32)
            nc.vector.memset(ones_row[:], 1.0)
            ones_rowb = cpool.tile([1, P], BF16)
            nc.vector.memset(ones_rowb[:], 1.0)
            ones_col = cpool.tile([P, 1], F32)
            nc.vector.memset(ones_col[:], 1.0)

            edge_consts = (identb, rowvals_b, ones_col)

            # att rows -> [P, HC] broadcast tiles (outer product with ones)
            with tc.tile_pool(name="p0ps", bufs=2, space="PSUM") as p0ps:
                att1_row = cpool.tile([1, HC], BF16)
                nc.sync.dma_start(att1_row[:], b16("att1", (HC,)))
                att2_row = cpool.tile([1, HC], BF16)
                nc.sync.dma_start(att2_row[:], b16("att2", (HC,)))
                ps_a1 = p0ps.tile([P, HC], F32, space="PSUM", tag="a")
                nc.tensor.matmul(ps_a1[:], lhsT=ones_rowb[:], rhs=att1_row[:],
                                 start=True, stop=True)
                att1_bc = cpool.tile([P, HC], BF16)
                nc.scalar.copy(att1_bc[:], ps_a1[:])
                ps_a2 = p0ps.tile([P, HC], F32, space="PSUM", tag="a")
                nc.tensor.matmul(ps_a2[:], lhsT=ones_rowb[:], rhs=att2_row[:],
                                 start=True, stop=True)
                att2_bc = cpool.tile([P, HC], BF16)
                nc.scalar.copy(att2_bc[:], ps_a2[:])

            # ---- P1: layer-1 node transforms (own shard only) -----------
            with (
                tc.tile_pool(name="p1sb", bufs=3) as p1sb,
                tc.tile_pool(name="p1ps", bufs=4, space="PSUM") as p1ps,
            ):
                W1l = p1sb.tile([F_IN + 1, HC], BF16, bufs=1)
                nc.sync.dma_start(W1l[:], b16("W1l", (F_IN + 1, HC)))
                W1r = p1sb.tile([F_IN + 1, HC], BF16, bufs=1)
                nc.sync.dma_start(W1r[:], b16("W1r", (F_IN + 1, HC)))
                xTq_o = B16_OFF["xTq"]
                for j in range(NBLK_CORE):
                    xtq = p1sb.tile([F_IN + 1, P], BF16, tag="xtq")
                    nc.sync.dma_start(
                        xtq[:], bass.AP(b16_d, xTq_o + j * P,
                                        [[NSHARD, F_IN + 1], [1, P]]))
                    ps = p1ps.tile([P, HC], F32, space="PSUM", tag="p1")
                    nc.tensor.matmul(ps[:], lhsT=xtq[:], rhs=W1l[:],
                                     start=True, stop=True)
                    ev = p1sb.tile([P, HC], BF16, tag="ev")
                    nc.scalar.copy(ev[:], ps[:])
                    nc.sync.dma_start(am1s_d[j * P:(j + 1) * P, :], ev[:])
                    ps2 = p1ps.tile([P, HC], F32, space="PSUM", tag="p1")
                    nc.tensor.matmul(ps2[:], lhsT=xtq[:], rhs=W1r[:],
                                     start=True, stop=True)
                    ev2 = p1sb.tile([P, HC], BF16, tag="ev2")
                    nc.scalar.copy(ev2[:], ps2[:])
                    nc.sync.dma_start(xr1_d[j * P:(j + 1) * P, :], ev2[:])

            # gather the full layer-1 source-transform table
            _collective("AllGather", OP.bypass, groups, [am1s_d[:]], [am1_d[:]])

            # ---- P2: layer-1 edge aggregation ---------------------------
            with (
                tc.tile_pool(name="e1sb", bufs=6) as esb,
                tc.tile_pool(name="e1psA", bufs=3, space="PSUM") as psA,
                tc.tile_pool(name="e1psT", bufs=1, space="PSUM") as psT,
                tc.tile_pool(name="e1psC", bufs=1, space="PSUM") as psC,
                tc.tile_pool(name="e1psD", bufs=1, space="PSUM") as psD,
                tc.tile_pool(name="e1psBN", bufs=1, space="PSUM") as psBN,
            ):
                psBN_sum = psBN.tile([1, HC], F32, space="PSUM")
                psBN_sq = psBN.tile([1, HC], F32, space="PSUM")
                _edge_layer(nc, (esb, psA, psT, psC, psD), t_max, am1_d,
                            xr1_d, att1_bc, h1raw_d, src_idx_d, dstl_aps,
                            edge_consts, psBN_sum, psBN_sq)

                # ---- P3: BN1 stats + scale/shift ------------------------
                scale1_bc, shift1_bc = _bn_scale_shift(
                    nc, hold, esb, psA, bn1in_d, bn1out_d, psBN_sum, psBN_sq,
                    f32("bn1g", (HC,)), f32("bn1b", (HC,)),
                    f32("bias1", (HC,)), ones_row, "b1", _collective)

            # ---- P4: BN1 apply + relu + build hT ------------------------
            with (
                tc.tile_pool(name="p4sb", bufs=3) as p4sb,
                tc.tile_pool(name="p4ps", bufs=2, space="PSUM") as p4ps,
            ):
                for j in range(NBLK_CORE):
                    raw = p4sb.tile([P, HC], F32, tag="raw")
                    nc.sync.dma_start(raw[:], h1raw_d[j * P:(j + 1) * P, :])
                    t1 = p4sb.tile([P, HC], F32, tag="t1")
                    nc.vector.tensor_mul(t1[:], raw[:], scale1_bc[:])
                    t2 = p4sb.tile([P, HC], F32, tag="t2")
                    nc.vector.tensor_add(t2[:], t1[:], shift1_bc[:])
                    hsb = p4sb.tile([P, HC], F32, tag="h")
                    nc.vector.tensor_scalar_max(hsb[:], t2[:], 0.0)
                    hb = p4sb.tile([P, HC], BF16, tag="hb")
                    nc.scalar.copy(hb[:], hsb[:])
                    pst = p4ps.tile([P, HC], BF16, space="PSUM", tag="tr")
                    for ch in range(4):
                        nc.tensor.transpose(pst[:, ch * P:(ch + 1) * P],
                                            hb[:, ch * P:(ch + 1) * P],
                                            identb[:])
                    ev4 = p4sb.tile([P, HC], BF16, tag="ev4")
                    nc.scalar.copy(ev4[:], pst[:])
                    for ch in range(4):
                        nc.sync.dma_start(
                            hT_d[ch * P:(ch + 1) * P, j * P:(j + 1) * P],
                            ev4[:, ch * P:(ch + 1) * P])

            # ---- P5: layer-2 node transforms ----------------------------
            with (
                tc.tile_pool(name="p5sb", bufs=3) as p5sb,
                tc.tile_pool(name="p5w", bufs=1) as p5w,
                tc.tile_pool(name="p5ps", bufs=4, space="PSUM") as p5ps,
            ):
                W2l_sb = [p5w.tile([P, HC], BF16, name=f"W2l{k}", tag=f"W2l{k}")
                          for k in range(4)]
                W2r_sb = [p5w.tile([P, HC], BF16, name=f"W2r{k}", tag=f"W2r{k}")
                          for k in range(4)]
                for k in range(4):
                    nc.sync.dma_start(W2l_sb[k][:],
                                      W2l_full_d[k * P:(k + 1) * P, :])
                    nc.sync.dma_start(W2r_sb[k][:],
                                      W2r_full_d[k * P:(k + 1) * P, :])
                b2l = p5w.tile([1, HC], BF16)
                nc.sync.dma_start(b2l[:], b16("b2l", (HC,)))
                b2r = p5w.tile([1, HC], BF16)
                nc.sync.dma_start(b2r[:], b16("b2r", (HC,)))
                for j in range(NBLK_CORE):
                    hTj = []
                    for k in range(4):
                        hx = p5sb.tile([P, P], BF16, tag=f"hTj{k}",
                                       name=f"hTj{k}")
                        nc.sync.dma_start(
                            hx[:], hT_d[k * P:(k + 1) * P, j * P:(j + 1) * P])
                        hTj.append(hx)
                    psl = p5ps.tile([P, HC], F32, space="PSUM", tag="l")
                    for k in range(4):
                        nc.tensor.matmul(psl[:], lhsT=hTj[k][:],
                                         rhs=W2l_sb[k][:], start=(k == 0),
                                         stop=False)
                    nc.tensor.matmul(psl[:], lhsT=ones_rowb[:], rhs=b2l[:],
                                     start=False, stop=True)
                    ev = p5sb.tile([P, HC], BF16, tag="ev")
                    nc.scalar.copy(ev[:], psl[:])
                    nc.sync.dma_start(am2s_d[j * P:(j + 1) * P, :], ev[:])
                    psr = p5ps.tile([P, HC], F32, space="PSUM", tag="r")
                    for k in range(4):
                        nc.tensor.matmul(psr[:], lhsT=hTj[k][:],
                                         rhs=W2r_sb[k][:], start=(k == 0),
                                         stop=False)
                    nc.tensor.matmul(psr[:], lhsT=ones_rowb[:], rhs=b2r[:],
                                     start=False, stop=True)
                    ev5 = p5sb.tile([P, HC], BF16, tag="ev5")
                    nc.scalar.copy(ev5[:], psr[:])
                    nc.sync.dma_start(xr2_d[j * P:(j + 1) * P, :], ev5[:])

            # ---- P6: AllGather layer-2 source transforms ----------------
            _collective("AllGather", OP.bypass, groups, [am2s_d[:]], [am2_d[:]])

            # ---- P7: layer-2 edge aggregation ---------------------------
            with (
                tc.tile_pool(name="e2sb", bufs=6) as esb,
                tc.tile_pool(name="e2psA", bufs=3, space="PSUM") as psA,
                tc.tile_pool(name="e2psT", bufs=1, space="PSUM") as psT,
                tc.tile_pool(name="e2psC", bufs=1, space="PSUM") as psC,
                tc.tile_pool(name="e2psD", bufs=1, space="PSUM") as psD,
                tc.tile_pool(name="e2psBN", bufs=1, space="PSUM") as psBN,
            ):
                psBN_sum = psBN.tile([1, HC], F32, space="PSUM")
                psBN_sq = psBN.tile([1, HC], F32, space="PSUM")
                _edge_layer(nc, (esb, psA, psT, psC, psD), t_max, am2_d,
                            xr2_d, att2_bc, h2raw_d, src_idx_d, dstl_aps,
                            edge_consts, psBN_sum, psBN_sq)
                scale2_bc, shift2_bc = _bn_scale_shift(
                    nc, hold, esb, psA, bn2in_d, bn2out_d, psBN_sum, psBN_sq,
                    f32("bn2g", (HC,)), f32("bn2b", (HC,)),
                    f32("bias2", (HC,)), ones_row, "b2", _collective)

            # ---- P8: BN2 apply + relu + pooling -------------------------
            with (
                tc.tile_pool(name="p8sb", bufs=3) as p8sb,
                tc.tile_pool(name="p8ps", bufs=1, space="PSUM") as p8ps,
            ):
                pool_ps = [p8ps.tile([P, B], F32, space="PSUM",
                                     name=f"pool{k}", tag=f"pool{k}")
                           for k in range(4)]
                for j in range(NBLK_CORE):
                    raw = p8sb.tile([P, HC], F32, tag="raw")
                    nc.sync.dma_start(raw[:], h2raw_d[j * P:(j + 1) * P, :])
                    t1 = p8sb.tile([P, HC], F32, tag="t1")
                    nc.vector.tensor_mul(t1[:], raw[:], scale2_bc[:])
                    t2 = p8sb.tile([P, HC], F32, tag="t2")
                    nc.vector.tensor_add(t2[:], t1[:], shift2_bc[:])
                    hsb = p8sb.tile([P, HC], F32, tag="h")
                    nc.vector.tensor_scalar_max(hsb[:], t2[:], 0.0)
                    bat8 = p8sb.tile([P, 1], U8, tag="bat8")
                    nc.sync.dma_start(bat8[:], batch_aps[j])
                    batch_sb = p8sb.tile([P, 1], F32, tag="bat")
                    nc.vector.tensor_copy(batch_sb[:], bat8[:])
                    ohb = p8sb.tile([P, B], F32, tag="ohb")
                    nc.vector.tensor_scalar(ohb[:], rowvals_f[:, :B],
                                            batch_sb[:], None, OP.is_equal)
                    for ch in range(4):
                        nc.tensor.matmul(pool_ps[ch][:],
                                         lhsT=hsb[:, ch * P:(ch + 1) * P],
                                         rhs=ohb[:], start=(j == 0),
                                         stop=(j == NBLK_CORE - 1))
                poolsb = p8sb.tile([P, 4 * B], F32)
                for ch in range(4):
                    nc.scalar.copy(poolsb[:, ch * B:(ch + 1) * B],
                                   pool_ps[ch][:])
                for ch in range(4):
                    nc.sync.dma_start(poolin_d[ch],
                                      poolsb[:, ch * B:(ch + 1) * B])
                _collective("AllReduce", OP.add, groups,
                            [poolin_d[:]], [poolout_d[:]])

            # ---- P9: head (bf16 fc1 weights, f32 accumulation) ----------
            with (
                tc.tile_pool(name="p9sb", bufs=1) as p9sb,
                tc.tile_pool(name="p9ps", bufs=1, space="PSUM") as p9ps,
            ):
                ci = p9sb.tile([1, B], F32)
                nc.sync.dma_start(ci[:], f32("cntinv", (B,)))
                ps_ci = p9ps.tile([P, B], F32, space="PSUM", tag="ci")
                nc.tensor.matmul(ps_ci[:], lhsT=ones_row[:], rhs=ci[:],
                                 start=True, stop=True)
                cib = p9sb.tile([P, B], F32)
                nc.scalar.copy(cib[:], ps_ci[:])

                zc = []
                for ch in range(4):
                    pc = p9sb.tile([P, B], F32, tag=f"pc{ch}")
                    nc.sync.dma_start(pc[:], poolout_d[ch])
                    z = p9sb.tile([P, B], F32, tag=f"z{ch}")
                    nc.vector.tensor_mul(z[:], pc[:], cib[:])
                    zc.append(z)
                gfT = p9sb.tile([G_DIM, B], F32)
                nc.sync.dma_start(gfT[:], f32("gfT", (G_DIM, B)))
                fc1 = []
                for ch in range(4):
                    w = p9sb.tile([P, C], F32, tag=f"w{ch}")
                    nc.sync.dma_start(w[:], fc1_full_d[ch * P:(ch + 1) * P, :])
                    fc1.append(w)
                fc1g = p9sb.tile([G_DIM, C], F32)
                nc.sync.dma_start(fc1g[:], fc1_full_d[HC:HC + G_DIM, :])
                fc1b = p9sb.tile([1, C], F32)
                nc.sync.dma_start(fc1b[:],
                                  fc1_full_d[HC + G_DIM:HC + G_DIM + 1, :])

                ps_z1 = p9ps.tile([B, C], F32, space="PSUM", tag="z1")
                for ch in range(4):
                    nc.tensor.matmul(ps_z1[:], lhsT=zc[ch][:], rhs=fc1[ch][:],
                                     start=(ch == 0), stop=False)
                nc.tensor.matmul(ps_z1[:], lhsT=gfT[:], rhs=fc1g[:],
                                 start=False, stop=False)
                nc.tensor.matmul(ps_z1[:], lhsT=ones_row[:, :B], rhs=fc1b[:],
                                 start=False, stop=True)
                z1 = p9sb.tile([B, C], F32)
                nc.scalar.activation(z1[:], ps_z1[:], AF.Relu)

                ps_z1T = p9ps.tile([C, B], F32, space="PSUM", tag="z1T")
                nc.tensor.transpose(ps_z1T[:], z1[:], ident[:B, :B])
                z1T = p9sb.tile([C, B], F32)
                nc.scalar.copy(z1T[:], ps_z1T[:])

                fc2 = p9sb.tile([C, 1], F32)
                nc.sync.dma_start(fc2[:], f32("fc2w", (C, 1)))
                ps_o = p9ps.tile([B, 1], F32, space="PSUM", tag="o")
                nc.tensor.matmul(ps_o[:], lhsT=z1T[:], rhs=fc2[:], start=True,
                                 stop=True)
                fc2b = p9sb.tile([B, 1], F32)
                nc.sync.dma_start(fc2b[:], f32("fc2b", (B, 1)))
                osb = p9sb.tile([B, 1], F32)
                nc.vector.tensor_scalar_add(osb[:], ps_o[:], fc2b[:])
                nc.sync.dma_start(out_d[:], osb[:])

    nc.compile()
    return nc


def _make_runner(nc):
    """Jit the PJRT executable once; returns run(in_maps) -> out_final[B]."""
    install_neuronx_cc_hook()
    partition_name = (nc.partition_id_tensor.name
                      if nc.partition_id_tensor else None)
    in_names, out_names, out_avals, zero_outs = [], [], [], []
    for alloc in nc.m.functions[0].allocations:
        if not isinstance(alloc, mybir.MemoryLocationSet):
            continue
        name = alloc.memorylocations[0].name
        if alloc.kind == "ExternalInput":
            if name != partition_name:
                in_names.append(name)
        elif alloc.kind == "ExternalOutput":
            out_names.append(name)
            shape = tuple(alloc.tensor_shape)
            dtype = mybir.dt.np(alloc.dtype)
            out_avals.append(jax.core.ShapedArray(shape, dtype))
            zero_outs.append(np.zeros(shape, dtype))
    n_params = len(in_names)
    n_outs = len(out_avals)
    in_names.extend(out_names)
    if partition_name is not None:
        in_names.append(partition_name)
    donate = tuple(range(n_params, n_params + n_outs))

    def _body(*args):
        operands = list(args)
        if partition_name is not None:
            operands.append(partition_id_tensor())
        return tuple(_bass_exec_p.bind(
            *operands, out_avals=tuple(out_avals), in_names=tuple(in_names),
            out_names=tuple(out_names), lowering_input_output_aliases=(),
            sim_require_finite=True, sim_require_nnan=True, nc=nc))

    devices = jax.devices()[:N_CORES]
    mesh = Mesh(np.asarray(devices), ("core",))
    in_specs = (PartitionSpec("core"),) * (n_params + n_outs)
    out_specs = (PartitionSpec("core"),) * len(out_names)
    sharded = jax.jit(
        shard_map(_body, mesh=mesh, in_specs=in_specs, out_specs=out_specs,
                  check_rep=False),
        donate_argnums=donate, keep_unused=True)
    param_names = in_names[:n_params]

    def run(in_maps):
        concat_in = [
            np.concatenate([np.asarray(m[name]) for m in in_maps], axis=0)
            for name in param_names
        ]
        concat_zeros = [
            np.zeros((N_CORES * z.shape[0], *z.shape[1:]), z.dtype)
            for z in zero_outs
        ]
        out_arrs = sharded(*concat_in, *concat_zeros)
        oi = out_names.index("out_final")
        return np.asarray(out_arrs[oi])[:B].reshape(B)

    return run


def _preprocess(inputs):
    """Host-side: edge grouping/padding + weight repacking into blobs."""
    x = np.asarray(inputs["x"], np.float32)
    gf = np.asarray(inputs["global_feat"], np.float32)
    ei = np.asarray(inputs["edge_index"])
    batch = np.asarray(inputs["batch"]).astype(np.int64)

    loops = np.arange(N, dtype=np.int32)
    src = np.concatenate([ei[0].astype(np.int32, copy=False), loops])
    dst = np.concatenate([ei[1].astype(np.int32, copy=False), loops])
    blk16 = (dst >> 7).astype(np.int16)
    order = np.argsort(blk16, kind="stable")  # radix sort on int16 keys
    src, dst = src[order], dst[order]
    blk = dst >> 7
    counts = np.bincount(blk, minlength=NBLK)
    t_max = max(1, int(np.ceil(counts.max() / P)))
    e_cap = t_max * P

    starts = np.concatenate([[0], np.cumsum(counts)])
    pos = np.arange(dst.shape[0]) - starts[blk]
    src_pad = np.zeros((NBLK, e_cap), np.int16)
    src_pad[blk, pos] = src.astype(np.int16)
    dstl_pad = np.full((NBLK, e_cap), 200, np.uint8)
    dstl_pad[blk, pos] = (dst & 127).astype(np.uint8)
    # [blk, e_cap] -> [blk, 128, t_max] with edge e of tile t at [e, t]
    src_t = np.ascontiguousarray(
        src_pad.reshape(NBLK, t_max, P).transpose(0, 2, 1))
    dstl_t = np.ascontiguousarray(
        dstl_pad.reshape(NBLK, t_max, P).transpose(0, 2, 1))

    xT_aug = np.zeros((F_IN + 1, N_PAD), np.float32)
    xT_aug[:F_IN, :N] = x.T
    xT_aug[F_IN, :] = 1.0
    xT_aug = xT_aug.astype("bfloat16")

    def aug_w(w, bvec):
        return np.concatenate([np.asarray(w, np.float32),
                               np.asarray(bvec, np.float32)[None, :]],
                              0).astype("bfloat16")

    W1l_aug = aug_w(inputs["W1l"], inputs["b1l"])
    W1r_aug = aug_w(inputs["W1r"], inputs["b1r"])
    W2l = np.asarray(inputs["W2l"], np.float32).astype("bfloat16")
    W2r = np.asarray(inputs["W2r"], np.float32).astype("bfloat16")
    b2l = np.asarray(inputs["b2l"], np.float32).astype("bfloat16")
    b2r = np.asarray(inputs["b2r"], np.float32).astype("bfloat16")
    att1 = np.asarray(inputs["att1"], np.float32).reshape(HC).astype("bfloat16")
    att2 = np.asarray(inputs["att2"], np.float32).reshape(HC).astype("bfloat16")
    gfT_f = np.ascontiguousarray(gf.T).astype(np.float32)

    fc1_pad = np.zeros((FC1_PAD, C), np.float32)
    fc1_pad[:FC1_ROWS - 1] = np.asarray(inputs["fc1_w"], np.float32)
    fc1_pad[FC1_ROWS - 1] = np.asarray(inputs["fc1_b"], np.float32)

    cnt = np.bincount(batch, minlength=B).astype(np.float32)
    cntinv = 1.0 / np.maximum(cnt, 1.0)

    f32_common = np.concatenate([
        np.asarray(inputs["bn1_g"], np.float32).reshape(-1),
        np.asarray(inputs["bn1_b"], np.float32).reshape(-1),
        np.asarray(inputs["bias1"], np.float32).reshape(-1),
        np.asarray(inputs["bn2_g"], np.float32).reshape(-1),
        np.asarray(inputs["bn2_b"], np.float32).reshape(-1),
        np.asarray(inputs["bias2"], np.float32).reshape(-1),
        cntinv,
        np.asarray(inputs["fc2_w"], np.float32).reshape(-1),
        np.full(B, np.asarray(inputs["fc2_b"], np.float32).reshape(-1)[0],
                np.float32),
    ])

    batch_p = np.full(N_PAD, 255, np.uint8)
    batch_p[:N] = batch.astype(np.uint8)
    batch_blk = batch_p.reshape(NBLK, P)

    in_maps = []
    for c in range(N_CORES):
        lo, hi = c * NBLK_CORE, (c + 1) * NBLK_CORE
        b16_blob = np.concatenate([
            np.ascontiguousarray(xT_aug[:, lo * P:hi * P]).reshape(-1),
            W1l_aug.reshape(-1), W1r_aug.reshape(-1),
            W2l[c * W2_SH:(c + 1) * W2_SH].reshape(-1),
            W2r[c * W2_SH:(c + 1) * W2_SH].reshape(-1),
            b2l, b2r, att1, att2,
        ])
        u8_blob = np.concatenate([
            dstl_t[lo:hi].reshape(-1),
            batch_blk[lo:hi].reshape(-1),
        ])
        f32_blob = np.concatenate([
            f32_common,
            fc1_pad[c * FC1_SH:(c + 1) * FC1_SH].reshape(-1),
            gfT_f.reshape(-1),
        ])
        in_maps.append({
            "blob_b16": b16_blob,
            "blob_f32": f32_blob,
            "blob_u8": u8_blob,
            "src16": src_t[lo:hi],
        })
    return in_maps, t_max


class _FastResult:
    exec_time_ns = None


def _run(inputs, trace=False):
    in_maps, t_max = _preprocess(inputs)
    if t_max not in _PROGRAM_CACHE:
        nc = _build_program(t_max)
        runner = _make_runner(nc)
        _PROGRAM_CACHE[t_max] = (nc, runner)
    nc, runner = _PROGRAM_CACHE[t_max]
    if trace:
        res = run_bass_kernel_spmd(nc, in_maps, list(range(N_CORES)),
                                   trace=True)
        out = np.asarray(res.results[0]["out_final"], np.float32).reshape(B)
        return out, res
    out = runner(in_maps)
    return np.asarray(out, np.float32), _FastResult()


def kernel(**inputs) -> np.ndarray:
    out, _ = _run(inputs, trace=False)
    return out
